# revision 1
# baseline (speedup 1.0000x reference)
"""Trainium2 Bass kernel for nn_Encoder (S=4096, D=512, H=8, E=64).

Sharding: sequence-parallel over 8 cores. Each core computes the full K/V
(every query needs them) plus attention/MLP for its own 512 rows; the only
cross-core traffic is two 8-byte AllReduces for the global LayerNorm
statistics (the reference normalizes jointly over the whole [S, D] tensor).
The host concatenates the per-core row shards.

Per-core dataflow:
  - x^T tiles built with PE transposes; K^T [he, t] and V [t, he] follow as
    fp32r matmuls (two heads packed per 128-wide stationary), written to a
    DRAM scratch and streamed back during attention (SBUF can't hold both).
  - logits are computed transposed, L^T[t, q] = K^T-slice.T @ Q^T, so the
    Exp output is already the A@V moving operand; softmax denominators fall
    out of a ones-column appended to V (row 64 of the accumulator).
  - per-head tensors (Q^T, outH^T, own K^T/V^T) live at partitions 0..63
    with the head index on a free dim, so every matmul/DVE op sees matching
    base partitions.
  - the MLP uses h1^T = W1-slice.T @ out1^T so no intermediate needs an
    explicit transpose.
"""

import os

os.environ.setdefault("JAX_PLATFORMS", "axon")

import numpy as np

import concourse.bass as bass
import concourse.tile as tile
from concourse import mybir
from concourse.bass_utils import run_bass_kernel_spmd
from concourse.masks import make_identity

dt = mybir.dt
AF = mybir.ActivationFunctionType
ALU = mybir.AluOpType
AX = mybir.AxisListType

N_CORES = 8
S, D, H, E = 4096, 512, 8, 64
F = 4 * D          # 2048
R = S // N_CORES   # 512 rows per core
EPS = 1e-5
SCALE = 1.0 / float(np.sqrt(E))
INV_SD = 1.0 / float(S * D)


def split_waits(nc):
    """Walrus codegen allows only one sync-wait per HW instruction. Move
    extra waits onto single-wait NoOps inserted before, same engine queue."""
    import bass_rust

    n = 0
    for bb in nc.m.functions[0].blocks:
        new_list = []
        changed = False
        for ins in bb.instructions:
            si = ins.sync_info
            if si is not None and si.on_wait is not None and len(si.on_wait) > 1:
                waits = list(si.on_wait)
                for w in waits[:-1]:
                    nop = bass_rust.InstNoOp(name=f"I-xwait-{n}")
                    n += 1
                    nop.engine = ins.engine
                    nop.sync_info = bass_rust.SyncInfo(on_wait=[w], on_update=[])
                    nc.register_instruction(nop)
                    new_list.append(nop)
                si.on_wait = waits[-1:]
                ins.sync_info = si
                changed = True
            new_list.append(ins)
        if changed:
            bb.instructions = new_list
    return nc


def build_nc():
    import contextlib

    nc = bass.Bass("TRN2", debug=False, num_devices=N_CORES)
    f32, f32r = dt.float32, dt.float32r

    # ---- I/O ----------------------------------------------------------
    x_d = nc.dram_tensor("x", [S, D], f32, kind="ExternalInput").ap()
    Wq_d = nc.dram_tensor("Wq", [H, D, E], f32, kind="ExternalInput").ap()
    Wk_d = nc.dram_tensor("Wk", [H, D, E], f32, kind="ExternalInput").ap()
    Wv_d = nc.dram_tensor("Wv", [H, D, E], f32, kind="ExternalInput").ap()
    bq_d = nc.dram_tensor("bq", [H, E], f32, kind="ExternalInput").ap()
    bk_d = nc.dram_tensor("bk", [H, E], f32, kind="ExternalInput").ap()
    bv_d = nc.dram_tensor("bv", [H, E], f32, kind="ExternalInput").ap()
    Wo_d = nc.dram_tensor("Wo", [D, D], f32, kind="ExternalInput").ap()
    bo_d = nc.dram_tensor("bo", [D], f32, kind="ExternalInput").ap()
    W1_d = nc.dram_tensor("W1", [D, F], f32, kind="ExternalInput").ap()
    b1_d = nc.dram_tensor("b1", [F], f32, kind="ExternalInput").ap()
    W2_d = nc.dram_tensor("W2", [F, D], f32, kind="ExternalInput").ap()
    b2_d = nc.dram_tensor("b2", [D], f32, kind="ExternalInput").ap()
    xr_d = nc.dram_tensor("x_rows", [R, D], f32, kind="ExternalInput").ap()
    lng_d = nc.dram_tensor("ln_g_rows", [R, D], f32, kind="ExternalInput").ap()
    lnb_d = nc.dram_tensor("ln_b_rows", [R, D], f32, kind="ExternalInput").ap()

    fin_d = nc.dram_tensor("final_rows", [R, D], f32, kind="ExternalOutput").ap()
    kp_d = nc.dram_tensor("Kp_rows", [R, D], f32, kind="ExternalOutput").ap()
    vp_d = nc.dram_tensor("Vp_rows", [R, D], f32, kind="ExternalOutput").ap()

    # row index q = qc*128 + p everywhere
    xr_v = xr_d.rearrange("(c p) d -> p c d", p=128)
    lng_v = lng_d.rearrange("(c p) d -> p c d", p=128)
    lnb_v = lnb_d.rearrange("(c p) d -> p c d", p=128)
    fin_v = fin_d.rearrange("(c p) d -> p c d", p=128)
    kp_v = kp_d.rearrange("(c p) d -> p c d", p=128)
    vp_v = vp_d.rearrange("(c p) d -> p c d", p=128)

    with tile.TileContext(nc) as tc, contextlib.ExitStack() as ctx, \
            nc.allow_low_precision(reason="bf16 matmul operands, fp32 accumulate"):
        ep = ctx.enter_context
        bf16 = dt.bfloat16

        # ---- pools ----------------------------------------------------
        single = ep(tc.tile_pool(name="single", bufs=1))
        a8 = ep(tc.tile_pool(name="a8", bufs=2))        # xa / xrT / sq
        big8 = ep(tc.tile_pool(name="big8", bufs=4))    # xt -> W1
        c8x = ep(tc.tile_pool(name="c8x", bufs=5))      # Wq/Wk/Wv -> W2
        d16 = ep(tc.tile_pool(name="d16", bufs=2))      # KTo/VTo -> h1T
        c8 = ep(tc.tile_pool(name="c8", bufs=2))        # xro(z), out1(w), out1T, fin
        qt_p = ep(tc.tile_pool(name="qt", bufs=1))      # Q^T [64, 8, R]
        ot_p = ep(tc.tile_pool(name="ot", bufs=1))      # outH^T [64, 8, R]
        evac = ep(tc.tile_pool(name="evac", bufs=4))
        pexp_p = ep(tc.tile_pool(name="pexp", bufs=3))
        vps_p = ep(tc.tile_pool(name="vps", bufs=3))
        otr_p = ep(tc.tile_pool(name="otr", bufs=2))
        ln_p = ep(tc.tile_pool(name="ln", bufs=2))
        wk = ep(tc.tile_pool(name="wk", bufs=2))
        sq_p = ep(tc.tile_pool(name="sq", bufs=1))
        # psum: tag "mm" 2x2banks + tag "po" 4x1bank = 8 banks
        ps_mm = ep(tc.tile_pool(name="ps_mm", bufs=3, space="PSUM"))
        ps_po = ep(tc.tile_pool(name="ps_po", bufs=2, space="PSUM"))
        dram = ep(tc.tile_pool(name="dram", bufs=1, space="DRAM"))

        # DRAM scratch for K^T and V' (streamed back during attention)
        KT_dram = dram.tile([H, 64, S], bf16)             # [h, e, t]
        VP_dram = dram.tile([32, 128, H, E + 1], bf16)    # [chunk, t%128, h, e']
        xb_dram = dram.tile([S, D], bf16)                 # x cast to bf16
        xrb_dram = dram.tile([R, D], bf16)                # x own rows, bf16

        # ---- constants / small loads ---------------------------------
        ident = single.tile([128, 128], f32)
        make_identity(nc, ident[:])
        onesP = single.tile([128, 8], f32)
        nc.vector.memset(onesP[:], 1.0)
        ones1 = single.tile([1, 128], f32)
        nc.vector.memset(ones1[:], 1.0)
        ones_row = single.tile([1, 128], bf16)
        nc.vector.tensor_copy(ones_row[:], ones1[:])
        ones_row_r = single.tile([1, 128], f32r)
        nc.vector.tensor_copy(ones_row_r[:], ones1[:])
        ones8 = single.tile([128, 8], bf16)
        nc.vector.tensor_copy(ones8[:], onesP[:])

        # per-head bias at partitions 0..63 (Q^T path): [64 e, 8 h]
        bqs = single.tile([64, H], f32)
        nc.sync.dma_start(bqs[:], bq_d.rearrange("h e -> e h"))
        # packed-pair biases [(h%2)*64+e, h//2] for packed evacuations
        bks2 = single.tile([128, 4], f32)
        nc.sync.dma_start(bks2[:], bk_d.rearrange("(c h2) e -> (h2 e) c", h2=2))
        bvs2 = single.tile([128, 4], f32)
        nc.sync.dma_start(bvs2[:], bv_d.rearrange("(c h2) e -> (h2 e) c", h2=2))
        b1s = single.tile([128, 16], f32)
        nc.sync.dma_start(b1s[:], b1_d.rearrange("(c p) -> p c", p=128))
        bo_r = single.tile([1, D], bf16)
        b2_r = single.tile([1, D], bf16)
        nc.gpsimd.dma_start(bo_r[:], bo_d.rearrange("(o d) -> o d", o=1))
        nc.gpsimd.dma_start(b2_r[:], b2_d.rearrange("(o d) -> o d", o=1))
        bv_bc = single.tile([128, D], f32)
        bv_flat = bv_d.rearrange("h e -> (h e)")
        nc.gpsimd.dma_start(
            bv_bc[:],
            bass.AP(tensor=bv_flat.tensor, offset=bv_flat.offset,
                    ap=[[0, 128]] + [list(a) for a in bv_flat.ap]),
        )
        eps_t = single.tile([1, 1], f32)
        nc.vector.memset(eps_t[:], EPS)

        # Wo in per-head-row layout padded to 128 rows (bottom zeroed so a
        # K=128 contraction against zero-padded outH^T is exact)
        Wo_s = single.tile([128, H, D], bf16)
        nc.vector.memset(Wo_s[:], 0.0)
        nc.gpsimd.dma_start(Wo_s[0:64, :, :], Wo_d.rearrange("(h e) d -> e h d", e=E))
        # Wo packed by head pair: [p = (h%2)*64+e, h//2, dm]
        Wo_p = single.tile([128, 4, D], bf16)
        nc.gpsimd.dma_start(Wo_p[:], Wo_d.rearrange("(c h2 e) d -> (h2 e) c d", h2=2, e=E))

        # Wq/Wk/Wv as [p=d%128, dc, he] with he = (h//2)*128 + (h%2)*64 + e
        w_qkv = {}
        for name, wd in (("q", Wq_d), ("k", Wk_d), ("v", Wv_d)):
            t = c8x.tile([128, 4, D], bf16, tag="c8x")
            wv4 = wd.rearrange("h (dc p) e -> dc p h e", p=128)
            for dc in range(4):
                nc.gpsimd.dma_start(
                    t[:, dc, :].rearrange("p (h e) -> p h e", e=E), wv4[dc]
                )
            w_qkv[name] = t

        QT = qt_p.tile([128, H, R], bf16)           # Q^T + bq, zero-padded rows
        nc.vector.memset(QT[64:128, :, :], 0.0)
        kt_ring = []
        for j in range(6):
            kt_t = single.tile([128, 512], bf16, name=f"ktr{j}")
            nc.vector.memset(kt_t[:], 0.0)
            kt_ring.append(kt_t)
        xro = c8.tile([128, 4, D], f32, tag="c8")   # x own rows; becomes z
        nc.sync.dma_start(xro[:], xr_v)

        # ---- phase 1: x^T via cast + DMA-transpose -> K^T, V' --------
        for tt in range(8):
            nc.gpsimd.dma_start(
                xb_dram[tt * 512:(tt + 1) * 512, :],
                x_d[tt * 512:(tt + 1) * 512, :],
            )
        nc.gpsimd.dma_start(xrb_dram[:], xr_d[:])
        for tt in range(8):
            xt = big8.tile([128, 4, 512], bf16, tag="big8")  # [d%128, dc, tl]
            for dc in range(4):
                nc.sync.dma_start(
                    xt[:, dc, :],
                    xb_dram[tt * 512:(tt + 1) * 512, dc * 128:(dc + 1) * 128],
                    transpose=True,
                )
            for mc in range(4):
                pk = ps_mm.tile([128, 512], f32, tag="mm")
                for dc in range(4):
                    nc.tensor.matmul(
                        pk[:],
                        lhsT=w_qkv["k"][:, dc, mc * 128:(mc + 1) * 128],
                        rhs=xt[:, dc, :],
                        start=(dc == 0), stop=(dc == 3),
                    )
                ke = evac.tile([128, 512], bf16, tag="evac")
                nc.scalar.activation(
                    ke[:], pk[:], AF.Identity, bias=bks2[:, mc:mc + 1]
                )
                nc.sync.dma_start(
                    KT_dram[2 * mc, :, tt * 512:(tt + 1) * 512], ke[0:64, :]
                )
                nc.sync.dma_start(
                    KT_dram[2 * mc + 1, :, tt * 512:(tt + 1) * 512], ke[64:128, :]
                )
            for vc in range(4):
                pv = ps_mm.tile([128, 512], f32, tag="mm")
                for dc in range(4):
                    nc.tensor.matmul(
                        pv[:],
                        lhsT=xt[:, dc, vc * 128:(vc + 1) * 128],
                        rhs=w_qkv["v"][:, dc, :],
                        start=(dc == 0), stop=(dc == 3),
                    )
                ve = evac.tile([128, H, E + 1], bf16, tag="evac")
                nc.vector.tensor_tensor(
                    ve[:, :, 0:E],
                    pv[:].rearrange("p (h e) -> p h e", e=E),
                    bv_bc[:].rearrange("p (h e) -> p h e", e=E),
                    ALU.add,
                )
                nc.vector.tensor_copy(ve[:, :, E], ones8[:])
                nc.sync.dma_start(VP_dram[tt * 4 + vc], ve[:])

        # ---- own-rows x^T, then per-head Q^T ------------------------
        xrT = a8.tile([128, 4, R], bf16, tag="a8")
        for dc in range(4):
            nc.sync.dma_start(
                xrT[:, dc, :], xrb_dram[:, dc * 128:(dc + 1) * 128],
                transpose=True,
            )

        def own_proj_perhead(dst, w_t, bias_t):
            """dst[64, h, R] = (x_rows @ W[h])^T + b[h], per head."""
            for h in range(H):
                he_local = (h // 2) * 128 + (h % 2) * 64
                pq = ps_mm.tile([64, 512], f32, tag="mm")
                for dc in range(4):
                    nc.tensor.matmul(
                        pq[:],
                        lhsT=w_t[:, dc, he_local:he_local + 64],
                        rhs=xrT[:, dc, :],
                        start=(dc == 0), stop=(dc == 3),
                    )
                nc.scalar.activation(
                    dst[0:64, h, :], pq[:], AF.Identity, bias=bias_t[:, h:h + 1]
                )

        def own_proj_packed(dst, w_t, bias2_t):
            """dst[128, mc, R] = pair-packed (x_rows @ W)^T + b."""
            for mc in range(4):
                pq = ps_mm.tile([128, 512], f32, tag="mm")
                for dc in range(4):
                    nc.tensor.matmul(
                        pq[:],
                        lhsT=w_t[:, dc, mc * 128:(mc + 1) * 128],
                        rhs=xrT[:, dc, :],
                        start=(dc == 0), stop=(dc == 3),
                    )
                nc.scalar.activation(
                    dst[:, mc, :], pq[:], AF.Identity, bias=bias2_t[:, mc:mc + 1]
                )

        def wo_project_packed(src_T, out_view):
            """out_view rows = concat_h(src) @ Wo + bo (src packed [128,4,R])."""
            for qc in range(4):
                po = ps_mm.tile([128, 512], f32, tag="mm")
                for mc in range(4):
                    nc.tensor.matmul(
                        po[:],
                        lhsT=src_T[:, mc, qc * 128:(qc + 1) * 128],
                        rhs=Wo_p[:, mc, :],
                        start=(mc == 0), stop=False,
                    )
                nc.tensor.matmul(
                    po[:], lhsT=ones_row[:], rhs=bo_r[:], start=False, stop=True
                )
                ot = evac.tile([128, 512], f32, tag="evac")
                nc.vector.tensor_copy(ot[:], po[:])
                nc.sync.dma_start(out_view[:, qc, :], ot[:])

        own_proj_perhead(QT, w_qkv["q"], bqs)

        # ---- phase 2: attention (4 passes x 2 heads, skewed AV) ------
        OT = ot_p.tile([128, H, R], bf16)  # normalized outH^T, zero-padded
        nc.vector.memset(OT[64:128, :, :], 0.0)
        kt_i = 0
        for pass_ in range(4):
            h0, h1 = 2 * pass_, 2 * pass_ + 1
            po_a = ps_po.tile([E + 1, R], f32, tag="po")
            po_b = ps_po.tile([E + 1, R], f32, tag="po")
            pend = None  # (vf, pexp, ch)
            for g in range(8):
                kt_a = kt_ring[kt_i % 6]
                kt_i += 1
                nc.sync.dma_start(kt_a[0:64, :], KT_dram[h0, :, g * 512:(g + 1) * 512])
                kt_b = kt_ring[kt_i % 6]
                kt_i += 1
                nc.sync.dma_start(kt_b[0:64, :], KT_dram[h1, :, g * 512:(g + 1) * 512])
                for cc in range(4):
                    ch = g * 4 + cc
                    vf = vps_p.tile([128, H, E + 1], bf16, tag="vps")
                    nc.sync.dma_start(vf[:], VP_dram[ch])
                    pl = ps_mm.tile([128, 2, 512], f32, tag="mm")
                    nc.tensor.matmul(
                        pl[:, 0, :],
                        lhsT=kt_a[:, cc * 128:(cc + 1) * 128],
                        rhs=QT[:, h0, :], start=True, stop=True,
                    )
                    nc.tensor.matmul(
                        pl[:, 1, :],
                        lhsT=kt_b[:, cc * 128:(cc + 1) * 128],
                        rhs=QT[:, h1, :], start=True, stop=True,
                    )
                    pexp = pexp_p.tile([128, 2, 512], bf16, tag="pexp")
                    nc.scalar.activation(pexp[:], pl[:], AF.Exp, scale=SCALE)
                    if pend is not None:
                        pvf, ppexp, pch = pend
                        nc.tensor.matmul(
                            po_a[:], lhsT=pvf[:, h0, :], rhs=ppexp[:, 0, :],
                            start=(pch == 0), stop=False,
                        )
                        nc.tensor.matmul(
                            po_b[:], lhsT=pvf[:, h1, :], rhs=ppexp[:, 1, :],
                            start=(pch == 0), stop=False,
                        )
                    pend = (vf, pexp, ch)
            pvf, ppexp, pch = pend
            nc.tensor.matmul(
                po_a[:], lhsT=pvf[:, h0, :], rhs=ppexp[:, 0, :],
                start=False, stop=True,
            )
            nc.tensor.matmul(
                po_b[:], lhsT=pvf[:, h1, :], rhs=ppexp[:, 1, :],
                start=False, stop=True,
            )
            # normalize rows 0..63 by the ones-column row 64
            for po_t, h in ((po_a, h0), (po_b, h1)):
                otr = otr_p.tile([E + 1, R], f32, tag="otr")
                nc.scalar.copy(otr[:], po_t[:])
                rden = otr_p.tile([1, R], f32r, tag="rden")
                nc.vector.reciprocal(rden[:], otr[E:E + 1, :])
                pb = ps_mm.tile([E, R], f32, tag="mm")
                nc.tensor.matmul(
                    pb[:], lhsT=ones_row_r[:, 0:E], rhs=rden[:],
                    start=True, stop=True,
                )
                nc.vector.tensor_tensor(OT[0:64, h, :], otr[0:E, :], pb[:], ALU.mult)

        # ---- phase 3: out proj + residual + global LN1 ---------------
        z = xro  # in place: z = x + out
        for qc in range(4):
            po = ps_mm.tile([128, 512], f32, tag="mm")
            for h in range(H):
                nc.tensor.matmul(
                    po[:],
                    lhsT=OT[:, h, qc * 128:(qc + 1) * 128],
                    rhs=Wo_s[:, h, :],
                    start=(h == 0), stop=False,
                )
            nc.tensor.matmul(
                po[:], lhsT=ones_row[:], rhs=bo_r[:], start=False, stop=True
            )
            nc.vector.tensor_tensor(z[:, qc, :], po[:], xro[:, qc, :], ALU.add)

        def stats_start(src_t, tag):
            """Partial [sum, sumsq] -> AllReduce; returns output dram tile."""
            sums = wk.tile([128, 2], f32, tag=f"sums{tag}")
            nc.vector.tensor_reduce(
                out=sums[:, 0:1], in_=src_t[:], axis=AX.XY, op=ALU.add
            )
            sq = sq_p.tile([128, 4, D], f32, tag="sq")
            nc.scalar.activation(
                sq[:], src_t[:], AF.Square, accum_out=sums[:, 1:2]
            )
            pr = ps_po.tile([1, 2], f32, tag="po")
            nc.tensor.matmul(
                pr[:], lhsT=onesP[:, 0:1], rhs=sums[:], start=True, stop=True
            )
            part = wk.tile([1, 2], f32, tag=f"part{tag}")
            nc.vector.tensor_copy(part[:], pr[:])
            cin = dram.tile([1, 2], f32)
            cout = dram.tile([1, 2], f32)
            nc.sync.dma_start(cin[:], part[:])
            nc.gpsimd.collective_compute(
                "AllReduce", ALU.add,
                replica_groups=[list(range(N_CORES))],
                ins=[cin[:]], outs=[cout[:]],
            )
            return cout

        def stats_finish(cout, tag):
            """-> [128, 2] sbuf tile: [:,0]=rstd, [:,1]=-mu*rstd (global)."""
            tot = wk.tile([1, 2], f32, tag=f"tot{tag}")
            nc.sync.dma_start(tot[:], cout[:])
            sc = wk.tile([1, 6], f32, tag=f"sc{tag}")
            mu, m2 = sc[0:1, 0:1], sc[0:1, 1:2]
            nc.vector.tensor_scalar_mul(mu, tot[0:1, 0:1], INV_SD)
            nc.vector.tensor_scalar_mul(m2, tot[0:1, 1:2], INV_SD)
            nc.vector.tensor_tensor(sc[0:1, 2:3], mu, mu, ALU.mult)
            nc.vector.tensor_tensor(sc[0:1, 3:4], m2, sc[0:1, 2:3], ALU.subtract)
            nc.scalar.activation(sc[0:1, 4:5], sc[0:1, 3:4], AF.Sqrt, bias=eps_t[:])
            st2 = wk.tile([1, 2], f32r, tag=f"st2{tag}")
            nc.vector.reciprocal(st2[0:1, 0:1], sc[0:1, 4:5])        # rstd
            nc.vector.tensor_tensor(sc[0:1, 5:6], mu, st2[0:1, 0:1], ALU.mult)
            nc.vector.tensor_scalar_mul(st2[0:1, 1:2], sc[0:1, 5:6], -1.0)
            pbc = ps_po.tile([128, 2], f32, tag="po")
            nc.tensor.matmul(pbc[:], lhsT=ones_row_r[:], rhs=st2[:],
                             start=True, stop=True)
            stb = wk.tile([128, 2], f32, tag=f"stb{tag}")
            nc.vector.tensor_copy(stb[:], pbc[:])
            return stb

        def ln_apply(dst_tile, src_t, stb, store_view=None):
            for qc in range(4):
                g_t = ln_p.tile([128, D], f32, tag="g")
                b_t = ln_p.tile([128, D], f32, tag="b")
                nc.sync.dma_start(g_t[:], lng_v[:, qc, :])
                nc.sync.dma_start(b_t[:], lnb_v[:, qc, :])
                n_t = evac.tile([128, D], f32, tag="evac")
                nc.scalar.activation(
                    n_t[:], src_t[:, qc, :], AF.Identity,
                    bias=stb[:, 1:2], scale=stb[:, 0:1],
                )
                nc.vector.tensor_tensor(n_t[:], n_t[:], g_t[:], ALU.mult)
                nc.vector.tensor_tensor(dst_tile[:, qc, :], n_t[:], b_t[:], ALU.add)
                if store_view is not None:
                    nc.sync.dma_start(store_view[:, qc, :], dst_tile[:, qc, :])

        cout1 = stats_start(z, "a")
        # Kp fills the first AllReduce's latency window
        KTo = d16.tile([128, 4, R], bf16, tag="d16")
        own_proj_packed(KTo, w_qkv["k"], bks2)
        wo_project_packed(KTo, kp_v)
        stb1 = stats_finish(cout1, "a")
        out1 = c8.tile([128, 4, D], f32, tag="c8")
        ln_apply(out1, z, stb1)
        out1T = c8.tile([128, 4, R], bf16, tag="c8")
        for dc in range(4):
            for qc in range(4):
                ptr = ps_po.tile([128, 128], f32, tag="po")
                nc.tensor.transpose(
                    ptr[:], out1[:, qc, dc * 128:(dc + 1) * 128], ident[:]
                )
                nc.vector.tensor_copy(out1T[:, dc, qc * 128:(qc + 1) * 128], ptr[:])

        # ---- phase 4: MLP + residual + global LN2 --------------------
        W1_v = W1_d.rearrange("(dc p) f -> dc p f", p=128)
        W1_s = []
        for j in range(4):
            t = big8.tile([128, F], bf16, tag="big8")
            nc.gpsimd.dma_start(t[:], W1_v[j])
            W1_s.append(t)
        W2_v = W2_d.rearrange("(g fc p) d -> g p fc d", p=128, fc=4)
        W2_s = []
        for j in range(4):
            t = c8x.tile([128, 4, D], bf16, tag="c8x")
            nc.gpsimd.dma_start(t[:], W2_v[j])
            W2_s.append(t)
        h1T = []
        for j in range(2):
            h1t_half = d16.tile([128, 8, R], bf16, tag="d16")
            h1T.append(h1t_half)
        for fm in range(16):
            ph = ps_mm.tile([128, R], f32, tag="mm")
            for dc in range(4):
                nc.tensor.matmul(
                    ph[:],
                    lhsT=W1_s[dc][:, fm * 128:(fm + 1) * 128],
                    rhs=out1T[:, dc, :],
                    start=(dc == 0), stop=(dc == 3),
                )
            nc.scalar.activation(
                h1T[fm // 8][:, fm % 8, :], ph[:], AF.Relu, bias=b1s[:, fm:fm + 1]
            )
        w = out1  # in place: w = out1 + out2
        for qc in range(4):
            po = ps_mm.tile([128, D], f32, tag="mm")
            for fm in range(16):
                nc.tensor.matmul(
                    po[:],
                    lhsT=h1T[fm // 8][:, fm % 8, qc * 128:(qc + 1) * 128],
                    rhs=W2_s[fm // 4][:, fm % 4, :],
                    start=(fm == 0), stop=False,
                )
            nc.tensor.matmul(
                po[:], lhsT=ones_row[:], rhs=b2_r[:], start=False, stop=True
            )
            nc.vector.tensor_tensor(w[:, qc, :], po[:], out1[:, qc, :], ALU.add)

        cout2 = stats_start(w, "b")
        # Vp fills the second AllReduce's latency window
        VTo = d16.tile([128, 4, R], bf16, tag="d16")
        own_proj_packed(VTo, w_qkv["v"], bvs2)
        wo_project_packed(VTo, vp_v)
        stb2 = stats_finish(cout2, "b")
        fin_s = c8.tile([128, 4, D], f32, tag="c8")
        ln_apply(fin_s, w, stb2, store_view=fin_v)

    split_waits(nc)
    return nc


_NC_CACHE = None


def _get_nc():
    global _NC_CACHE
    if _NC_CACHE is None:
        _NC_CACHE = build_nc()
    return _NC_CACHE


def kernel(**inputs):
    inp = {k: np.ascontiguousarray(np.asarray(v, dtype=np.float32))
           for k, v in inputs.items()}
    in_maps = []
    for c in range(N_CORES):
        rows = slice(c * R, (c + 1) * R)
        in_maps.append(dict(
            x=inp["x"], Wq=inp["Wq"], Wk=inp["Wk"], Wv=inp["Wv"],
            bq=inp["bq"], bk=inp["bk"], bv=inp["bv"],
            Wo=inp["Wo"], bo=inp["bo"], W1=inp["W1"], b1=inp["b1"],
            W2=inp["W2"], b2=inp["b2"],
            x_rows=inp["x"][rows],
            ln_g_rows=inp["ln_g"][rows], ln_b_rows=inp["ln_b"][rows],
        ))
    nc = _get_nc()
    res = run_bass_kernel_spmd(nc, in_maps, list(range(N_CORES)))
    final = np.concatenate([res.results[c]["final_rows"] for c in range(N_CORES)])
    Kp = np.concatenate([res.results[c]["Kp_rows"] for c in range(N_CORES)])
    Vp = np.concatenate([res.results[c]["Vp_rows"] for c in range(N_CORES)])
    return (final, Kp, Vp)



# revision 15
# speedup vs baseline: 1.4654x; 1.4654x over previous
"""Trainium2 Bass kernel for nn_Encoder (S=4096, D=512, H=8, E=64).

Sharding: sequence-parallel over 8 cores, but K/V are computed distributed:
each core projects Q/K/V only for its OWN 512 rows, then a single AllGather
(~1 MB/rank) assembles the full K^T and V' on every core. Attention, the
output projection, the global LayerNorms (two 8-byte AllReduces for the
joint [S, D] statistics) and the MLP then run on the own-row shard. The
host concatenates the per-core row shards.

Key differences vs the earlier replicated-KV version:
  - no replicated full-S K/V projection (was ~55us of PE per core)
  - K^T / V' live entirely in SBUF during attention (no DRAM scratch
    streaming; the old version re-read V' four times)
  - weights are cast to bf16 and pre-laid-out on the host, halving weight
    HBM traffic and removing all on-chip transpose/cast preludes
  - all per-head tensors use 64-partition tiles at base 0 (K=64 matmuls)
  - Kp fills the AllGather window; Vp + z-transposes fill the LN1
    AllReduce window.
"""

import os

os.environ.setdefault("JAX_PLATFORMS", "axon")

import numpy as np
import ml_dtypes

import concourse.bass as bass
import concourse.tile as tile
from concourse import mybir
from concourse.bass_utils import run_bass_kernel_spmd
from concourse.masks import make_identity

dt = mybir.dt
AF = mybir.ActivationFunctionType
ALU = mybir.AluOpType
AX = mybir.AxisListType

N_CORES = 8
S, D, H, E = 4096, 512, 8, 64
F = 4 * D          # 2048
R = S // N_CORES   # 512 rows per core
EPS = 1e-5
SCALE = 1.0 / float(np.sqrt(E))
INV_SD = 1.0 / float(S * D)

KT_ELEMS = 128 * 4 * 512          # KTo dump  [p, mc, t]
VP_ELEMS = 128 * 4 * 8 * 65       # VPo dump  [p, tc, h, e']
AG_ELEMS = KT_ELEMS + VP_ELEMS

BF16 = ml_dtypes.bfloat16


def split_waits(nc):
    """Walrus codegen allows only one sync-wait per HW instruction. Move
    extra waits onto single-wait NoOps inserted before, same engine queue."""
    import bass_rust

    n = 0
    for bb in nc.m.functions[0].blocks:
        new_list = []
        changed = False
        for ins in bb.instructions:
            si = ins.sync_info
            if si is not None and si.on_wait is not None and len(si.on_wait) > 1:
                waits = list(si.on_wait)
                for w in waits[:-1]:
                    nop = bass_rust.InstNoOp(name=f"I-xwait-{n}")
                    n += 1
                    nop.engine = ins.engine
                    nop.sync_info = bass_rust.SyncInfo(on_wait=[w], on_update=[])
                    nc.register_instruction(nop)
                    new_list.append(nop)
                si.on_wait = waits[-1:]
                ins.sync_info = si
                changed = True
            new_list.append(ins)
        if changed:
            bb.instructions = new_list
    return nc


def build_nc():
    import contextlib

    nc = bass.Bass("TRN2", debug=False, num_devices=N_CORES)
    f32, f32r, bf16 = dt.float32, dt.float32r, dt.bfloat16

    # ---- I/O (all host-prepped layouts) -------------------------------
    xro_d = nc.dram_tensor("xro", [128, 4, D], f32, kind="ExternalInput").ap()
    xrT_d = nc.dram_tensor("xrT", [128, 4, R], bf16, kind="ExternalInput").ap()
    wq_d = nc.dram_tensor("wq", [128, 4, D], bf16, kind="ExternalInput").ap()
    wk_d = nc.dram_tensor("wk", [128, 4, D], bf16, kind="ExternalInput").ap()
    wv_d = nc.dram_tensor("wv", [128, 4, D], bf16, kind="ExternalInput").ap()
    wos_d = nc.dram_tensor("wo_s", [64, H, D], bf16, kind="ExternalInput").ap()
    wop_d = nc.dram_tensor("wo_p", [128, 4, D], bf16, kind="ExternalInput").ap()
    w1_d = nc.dram_tensor("w1", [128, 4, F], bf16, kind="ExternalInput").ap()
    w2_d = nc.dram_tensor("w2", [128, 4, 4, D], bf16, kind="ExternalInput").ap()
    bqs_d = nc.dram_tensor("bqs2", [128, 4], f32, kind="ExternalInput").ap()
    bks2_d = nc.dram_tensor("bks2", [128, 4], f32, kind="ExternalInput").ap()
    bvs2_d = nc.dram_tensor("bvs2", [128, 4], f32, kind="ExternalInput").ap()
    bvbc_d = nc.dram_tensor("bv_bc", [128, D], f32, kind="ExternalInput").ap()
    b1s_d = nc.dram_tensor("b1s", [128, 16], f32, kind="ExternalInput").ap()
    bor_d = nc.dram_tensor("bo_r", [1, D], bf16, kind="ExternalInput").ap()
    b2r_d = nc.dram_tensor("b2_r", [1, D], bf16, kind="ExternalInput").ap()
    gnat_d = nc.dram_tensor("g_nat", [128, 4, D], bf16, kind="ExternalInput").ap()
    bnat_d = nc.dram_tensor("b_nat", [128, 4, D], bf16, kind="ExternalInput").ap()
    gT_d = nc.dram_tensor("gT", [128, 4, R], bf16, kind="ExternalInput").ap()
    bT_d = nc.dram_tensor("bT", [128, 4, R], bf16, kind="ExternalInput").ap()

    fin_d = nc.dram_tensor("final_rows", [R, D], f32, kind="ExternalOutput").ap()
    kp_d = nc.dram_tensor("Kp_rows", [R, D], f32, kind="ExternalOutput").ap()
    vp_d = nc.dram_tensor("Vp_rows", [R, D], f32, kind="ExternalOutput").ap()

    # row index q = qc*128 + p everywhere
    fin_v = fin_d.rearrange("(c p) d -> p c d", p=128)
    kp_v = kp_d.rearrange("(c p) d -> p c d", p=128)
    vp_v = vp_d.rearrange("(c p) d -> p c d", p=128)

    with tile.TileContext(nc) as tc, contextlib.ExitStack() as ctx, \
            nc.allow_low_precision(reason="bf16 matmul operands, fp32 accumulate"):
        ep = ctx.enter_context

        # ---- pools ----------------------------------------------------
        single = ep(tc.tile_pool(name="single", bufs=1))
        wpool = ep(tc.tile_pool(name="wpool", bufs=1))
        kt_p = ep(tc.tile_pool(name="ktp", bufs=2))      # K^T pair per pass, 1MB
        vp_p = ep(tc.tile_pool(name="vpp", bufs=8))      # V' per rank, 0.53MB
        pexp_p = ep(tc.tile_pool(name="pexp", bufs=3))
        evac = ep(tc.tile_pool(name="evac", bufs=3))
        otr_p = ep(tc.tile_pool(name="otr", bufs=2))
        wk_p = ep(tc.tile_pool(name="wk", bufs=2))
        # psum: tag "mm" 3x2banks + tag "po" 2x1bank = 8 banks
        ps_mm = ep(tc.tile_pool(name="ps_mm", bufs=3, space="PSUM"))
        ps_po = ep(tc.tile_pool(name="ps_po", bufs=2, space="PSUM"))
        dram = ep(tc.tile_pool(name="dram", bufs=1, space="DRAM"))

        # AllGather buffers
        ag_in = dram.tile([AG_ELEMS], bf16)
        ag_out = dram.tile([N_CORES, AG_ELEMS], bf16)
        agi_kt = ag_in[0:KT_ELEMS].rearrange("(p mc t) -> p mc t", p=128, mc=4)
        agi_vp = ag_in[KT_ELEMS:AG_ELEMS].rearrange(
            "(p tc h e) -> p tc h e", p=128, tc=4, h=H)
        # per-head K^T view across ranks: [64e, rank, t]
        ago_kt = ag_out[:, 0:KT_ELEMS].rearrange(
            "r (p mc t) -> p mc r t", p=128, mc=4)  # p = (h%2)*64+e, mc = h//2
        ago_vp = ag_out[:, KT_ELEMS:AG_ELEMS].rearrange(
            "r (p x) -> r p x", p=128)              # x = tc*520 + h*65 + e'

        # ---- constants ------------------------------------------------
        ident = single.tile([128, 128], f32)
        make_identity(nc, ident[:])
        ones1 = single.tile([1, 128], f32)
        nc.vector.memset(ones1[:], 1.0)
        ones_row = single.tile([1, 128], bf16)
        nc.vector.tensor_copy(ones_row[:], ones1[:])
        ones_row_r = single.tile([1, 128], f32r)
        nc.vector.tensor_copy(ones_row_r[:], ones1[:])
        ones8 = single.tile([128, 8], bf16)
        nc.vector.memset(ones8[:], 1.0)
        onesP = single.tile([128, 1], f32)
        nc.vector.memset(onesP[:], 1.0)
        eps_t = single.tile([1, 1], f32)
        nc.vector.memset(eps_t[:], EPS)

        # ---- small loads ----------------------------------------------
        bqs2 = single.tile([128, 4], f32)
        nc.sync.dma_start(bqs2[:], bqs_d)
        bks2 = single.tile([128, 4], f32)
        nc.sync.dma_start(bks2[:], bks2_d)
        bvs2 = single.tile([128, 4], f32)
        nc.sync.dma_start(bvs2[:], bvs2_d)
        bv_bc = single.tile([128, D], f32)
        nc.sync.dma_start(bv_bc[:], bvbc_d)
        b1s = single.tile([128, 16], f32)
        nc.sync.dma_start(b1s[:], b1s_d)
        bo_r = single.tile([1, D], bf16)
        nc.sync.dma_start(bo_r[:], bor_d)
        b2_r = single.tile([1, D], bf16)
        nc.sync.dma_start(b2_r[:], b2r_d)

        # ---- main loads -----------------------------------------------
        xro = single.tile([128, 4, D], f32)       # x own rows; becomes z
        nc.sync.dma_start(xro[:], xro_d)
        xrT = single.tile([128, 4, R], bf16)      # x^T own rows (host prepped)
        nc.sync.dma_start(xrT[:], xrT_d)
        w_q = wpool.tile([128, 4, D], bf16)
        nc.sync.dma_start(w_q[:], wq_d)
        w_k = wpool.tile([128, 4, D], bf16)
        nc.sync.dma_start(w_k[:], wk_d)
        w_v = wpool.tile([128, 4, D], bf16)
        nc.sync.dma_start(w_v[:], wv_d)
        Wo_p = wpool.tile([128, 4, D], bf16)
        nc.sync.dma_start(Wo_p[:], wop_d)

        # ---- phase 1: own-row projections -----------------------------
        def own_proj_packed(dst, w_t, bias2_t):
            """dst[128, mc, R] = pair-packed (x_rows @ W)^T + b, p=(h%2)*64+e."""
            for mc in range(4):
                pq = ps_mm.tile([128, R], f32, tag="mm")
                for dc in range(4):
                    nc.tensor.matmul(
                        pq[:],
                        lhsT=w_t[:, dc, mc * 128:(mc + 1) * 128],
                        rhs=xrT[:, dc, :],
                        start=(dc == 0), stop=(dc == 3),
                    )
                nc.vector.tensor_scalar_add(
                    dst[:, mc, :], pq[:], bias2_t[:, mc:mc + 1]
                )

        KTo = single.tile([128, 4, R], bf16)
        own_proj_packed(KTo, w_k, bks2)
        nc.sync.dma_start(agi_kt, KTo[:])

        VPo = single.tile([128, 4, H, E + 1], bf16)   # [t%128, tc, h, e']
        for vc in range(4):
            pv = ps_mm.tile([128, D], f32, tag="mm")
            for dc in range(4):
                nc.tensor.matmul(
                    pv[:],
                    lhsT=xrT[:, dc, vc * 128:(vc + 1) * 128],
                    rhs=w_v[:, dc, :],
                    start=(dc == 0), stop=(dc == 3),
                )
            nc.vector.tensor_tensor(
                VPo[:, vc, :, 0:E],
                pv[:].rearrange("p (h e) -> p h e", e=E),
                bv_bc[:].rearrange("p (h e) -> p h e", e=E),
                ALU.add,
            )
            nc.vector.tensor_copy(VPo[:, vc, :, E], ones8[:])
        nc.sync.dma_start(agi_vp, VPo[:])

        # ---- AllGather of K^T + V' (1.03 MB per rank) ----------------
        nc.gpsimd.collective_compute(
            "AllGather", ALU.bypass,
            replica_groups=[list(range(N_CORES))],
            ins=[ag_in[:]], outs=[ag_out[:]],
        )

        # ---- AG window: Q^T, V^T packed, Kp --------------------------
        # QT[:, h, :] holds Q_h^T at rows (h%2)*64..+64, zeros elsewhere, so
        # a pair-packed K^T block serves as shared lhsT for both heads.
        QT = single.tile([128, H, R], bf16)
        nc.vector.memset(QT[:], 0.0)
        for c in range(4):
            pq = ps_mm.tile([128, R], f32, tag="mm")
            for dc in range(4):
                nc.tensor.matmul(
                    pq[0:64, :],
                    lhsT=w_q[:, dc, c * 128:c * 128 + 64],
                    rhs=xrT[:, dc, :],
                    start=(dc == 0), stop=(dc == 3),
                )
            for dc in range(4):
                nc.tensor.matmul(
                    pq[64:128, :],
                    lhsT=w_q[:, dc, c * 128 + 64:c * 128 + 128],
                    rhs=xrT[:, dc, :],
                    start=(dc == 0), stop=(dc == 3),
                )
            nc.vector.tensor_scalar_add(
                QT[0:64, 2 * c, :], pq[0:64, :], bqs2[0:64, c:c + 1]
            )
            nc.vector.tensor_scalar_add(
                QT[64:128, 2 * c + 1, :], pq[64:128, :], bqs2[64:128, c:c + 1]
            )

        VTo = single.tile([128, 4, R], bf16)
        own_proj_packed(VTo, w_v, bvs2)

        def wo_project_packed(src_T, out_view):
            """out_view rows = concat_h(src) @ Wo + bo (src packed [128,4,R])."""
            for qc in range(4):
                po = ps_mm.tile([128, D], f32, tag="mm")
                for mc in range(4):
                    nc.tensor.matmul(
                        po[:],
                        lhsT=src_T[:, mc, qc * 128:(qc + 1) * 128],
                        rhs=Wo_p[:, mc, :],
                        start=(mc == 0), stop=False,
                    )
                nc.tensor.matmul(
                    po[:], lhsT=ones_row[:], rhs=bo_r[:], start=False, stop=True
                )
                ot = evac.tile([128, D], f32, tag="evac")
                nc.vector.tensor_copy(ot[:], po[:])
                nc.sync.dma_start(out_view[:, qc, :], ot[:])

        wo_project_packed(KTo, kp_v)    # Kp during the AG latency window

        # ---- attention readbacks -------------------------------------
        vp_r = []
        for r in range(N_CORES):
            t = vp_p.tile([128, 4, H, E + 1], bf16, tag="vpr", name=f"vpr{r}")
            nc.sync.dma_start(
                t[:].rearrange("p tc h e -> p (tc h e)"), ago_vp[r]
            )
            vp_r.append(t)

        # ---- phase 2: attention (4 passes x 2 heads) ------------------
        OT = single.tile([64, H, R], bf16)  # normalized outH^T
        w1_loaded = False
        W1_s = wpool.tile([128, 4, F], bf16)
        W2_s = wpool.tile([128, 4, 4, D], bf16)

        for pass_ in range(4):
            h0, h1 = 2 * pass_, 2 * pass_ + 1
            # pair-packed K^T for both heads: p = (h%2)*64 + e
            kt_t = kt_p.tile([128, N_CORES, R], bf16, tag="kt", name=f"kt{pass_}")
            nc.sync.dma_start(kt_t[:], ago_kt[:, pass_, :, :])
            po_a = ps_po.tile([E + 1, R], f32, tag="po")
            po_b = ps_po.tile([E + 1, R], f32, tag="po")
            for ch in range(32):
                r, tc = ch // 4, ch % 4
                pl = ps_mm.tile([128, 2, R], f32, tag="mm")
                nc.tensor.matmul(
                    pl[:, 0, :],
                    lhsT=kt_t[:, r, tc * 128:(tc + 1) * 128],
                    rhs=QT[:, h0, :], start=True, stop=True,
                )
                nc.tensor.matmul(
                    pl[:, 1, :],
                    lhsT=kt_t[:, r, tc * 128:(tc + 1) * 128],
                    rhs=QT[:, h1, :], start=True, stop=True,
                )
                pexp = pexp_p.tile([128, 2, R], bf16, tag="pexp")
                nc.scalar.activation(pexp[:], pl[:], AF.Exp, scale=SCALE)
                nc.tensor.matmul(
                    po_a[:], lhsT=vp_r[r][:, tc, h0, :], rhs=pexp[:, 0, :],
                    start=(ch == 0), stop=(ch == 31),
                )
                nc.tensor.matmul(
                    po_b[:], lhsT=vp_r[r][:, tc, h1, :], rhs=pexp[:, 1, :],
                    start=(ch == 0), stop=(ch == 31),
                )
            # normalize rows 0..63 by the ones-column row 64
            for po_t, h in ((po_a, h0), (po_b, h1)):
                otr = otr_p.tile([E + 1, R], f32, tag="otr")
                nc.vector.tensor_copy(otr[:], po_t[:])
                rden = otr_p.tile([1, R], f32r, tag="rden")
                nc.vector.reciprocal(rden[:], otr[E:E + 1, :])
                pb = ps_po.tile([E, R], f32, tag="po")
                nc.tensor.matmul(
                    pb[:], lhsT=ones_row_r[:, 0:E], rhs=rden[:],
                    start=True, stop=True,
                )
                nc.vector.tensor_tensor(OT[:, h, :], otr[0:E, :], pb[:], ALU.mult)
            if pass_ == 1 and not w1_loaded:
                # stream the MLP weights while attention still runs
                w1_loaded = True
                nc.sync.dma_start(W1_s[:], w1_d)
                nc.sync.dma_start(W2_s[:], w2_d)

        # ---- phase 3: out proj + residual + global LN1 ----------------
        # Wo_s reuses a freed K^T slot (same pool tag, disjoint lifetime)
        Wo_s = kt_p.tile([64, H, D], bf16, tag="kt", name="Wo_s")
        nc.sync.dma_start(Wo_s[:], wos_d)
        z = xro  # in place: z = x + out
        for qc in range(4):
            po = ps_mm.tile([128, D], f32, tag="mm")
            for h in range(H):
                nc.tensor.matmul(
                    po[:],
                    lhsT=OT[:, h, qc * 128:(qc + 1) * 128],
                    rhs=Wo_s[:, h, :],
                    start=(h == 0), stop=False,
                )
            nc.tensor.matmul(
                po[:], lhsT=ones_row[:], rhs=bo_r[:], start=False, stop=True
            )
            nc.vector.tensor_tensor(z[:, qc, :], po[:], xro[:, qc, :], ALU.add)

        def stats_start(src_t, tag):
            """Partial [sum, sumsq] -> AllGather (cheaper floor than
            AllReduce for 8 B); returns the gathered [8, 2] dram tile."""
            sums = wk_p.tile([128, 5], f32, tag=f"sums{tag}")
            nc.vector.tensor_reduce(
                out=sums[:, 0:1], in_=src_t[:], axis=AX.XY, op=ALU.add
            )
            for qc in range(4):
                sqv = evac.tile([128, D], f32, tag="evac")
                nc.scalar.activation(
                    sqv[:], src_t[:, qc, :], AF.Square,
                    accum_out=sums[:, 1 + qc:2 + qc],
                )
            pr = ps_po.tile([1, 5], f32, tag="po")
            nc.tensor.matmul(
                pr[:], lhsT=onesP[:], rhs=sums[:], start=True, stop=True
            )
            part = wk_p.tile([1, 2], f32, tag=f"part{tag}")
            nc.vector.tensor_copy(part[:, 0:1], pr[:, 0:1])
            nc.vector.tensor_reduce(
                out=part[:, 1:2], in_=pr[:, 1:5], axis=AX.X, op=ALU.add
            )
            cin = dram.tile([1, 2], f32)
            cout = dram.tile([N_CORES, 2], f32)
            nc.sync.dma_start(cin[:], part[:])
            nc.gpsimd.collective_compute(
                "AllGather", ALU.bypass,
                replica_groups=[list(range(N_CORES))],
                ins=[cin[:]], outs=[cout[:]],
            )
            return cout

        def stats_finish(cout, tag):
            """-> [128, 2] sbuf tile: [:,0]=rstd, [:,1]=-mu*rstd (global)."""
            tot8 = wk_p.tile([N_CORES, 2], f32, tag=f"tot8{tag}")
            nc.sync.dma_start(tot8[:], cout[:])
            pr8 = ps_po.tile([1, 2], f32, tag="po")
            nc.tensor.matmul(
                pr8[:], lhsT=onesP[0:N_CORES, :], rhs=tot8[:],
                start=True, stop=True,
            )
            tot = wk_p.tile([1, 2], f32, tag=f"tot{tag}")
            nc.vector.tensor_copy(tot[:], pr8[:])
            sc = wk_p.tile([1, 6], f32, tag=f"sc{tag}")
            mu, m2 = sc[0:1, 0:1], sc[0:1, 1:2]
            nc.vector.tensor_scalar_mul(mu, tot[0:1, 0:1], INV_SD)
            nc.vector.tensor_scalar_mul(m2, tot[0:1, 1:2], INV_SD)
            nc.vector.tensor_tensor(sc[0:1, 2:3], mu, mu, ALU.mult)
            nc.vector.tensor_tensor(sc[0:1, 3:4], m2, sc[0:1, 2:3], ALU.subtract)
            nc.scalar.activation(sc[0:1, 4:5], sc[0:1, 3:4], AF.Sqrt, bias=eps_t[:])
            st2 = wk_p.tile([1, 2], f32r, tag=f"st2{tag}")
            nc.vector.reciprocal(st2[0:1, 0:1], sc[0:1, 4:5])        # rstd
            nc.vector.tensor_tensor(sc[0:1, 5:6], mu, st2[0:1, 0:1], ALU.mult)
            nc.vector.tensor_scalar_mul(st2[0:1, 1:2], sc[0:1, 5:6], -1.0)
            pbc = ps_po.tile([128, 2], f32, tag="po")
            nc.tensor.matmul(pbc[:], lhsT=ones_row_r[:], rhs=st2[:],
                             start=True, stop=True)
            stb = wk_p.tile([128, 2], f32, tag=f"stb{tag}")
            nc.vector.tensor_copy(stb[:], pbc[:])
            return stb

        cout1 = stats_start(z, "a")
        # AR1 latency window: Vp projection + z^T transposes
        wo_project_packed(VTo, vp_v)
        zT = single.tile([128, 4, R], bf16)        # z^T for the MLP path
        for dc in range(4):
            for qc in range(4):
                ptr = ps_po.tile([128, 128], f32, tag="po")
                nc.tensor.transpose(
                    ptr[:], z[:, qc, dc * 128:(dc + 1) * 128], ident[:]
                )
                nc.vector.tensor_copy(zT[:, dc, qc * 128:(qc + 1) * 128], ptr[:])
        stb1 = stats_finish(cout1, "a")

        # LN1 in both orientations (natural for residual, ^T for the MLP)
        g_nat = single.tile([128, 4, D], bf16)
        nc.sync.dma_start(g_nat[:], gnat_d)
        b_nat = single.tile([128, 4, D], bf16)
        nc.sync.dma_start(b_nat[:], bnat_d)


        out1 = single.tile([128, 4, D], f32)
        for qc in range(4):
            n_t = evac.tile([128, D], f32, tag="evac")
            nc.scalar.activation(
                n_t[:], z[:, qc, :], AF.Identity,
                bias=stb1[:, 1:2], scale=stb1[:, 0:1],
            )
            nc.vector.tensor_tensor(n_t[:], n_t[:], g_nat[:, qc, :], ALU.mult)
            nc.vector.tensor_tensor(out1[:, qc, :], n_t[:], b_nat[:, qc, :], ALU.add)
        out1T = single.tile([128, 4, R], bf16)
        for dc in range(4):
            gT_t = evac.tile([128, R], bf16, tag="evacT")
            nc.sync.dma_start(gT_t[:], gT_d[:, dc, :])
            bT_t = evac.tile([128, R], bf16, tag="evacT")
            nc.sync.dma_start(bT_t[:], bT_d[:, dc, :])
            nT = evac.tile([128, R], bf16, tag="evacT")
            nc.scalar.activation(
                nT[:], zT[:, dc, :], AF.Identity,
                bias=stb1[:, 1:2], scale=stb1[:, 0:1],
            )
            nc.vector.tensor_tensor(nT[:], nT[:], gT_t[:], ALU.mult)
            nc.vector.tensor_tensor(out1T[:, dc, :], nT[:], bT_t[:], ALU.add)

        # ---- phase 4: MLP + residual + global LN2 --------------------
        h1T = single.tile([128, 16, R], bf16)
        for fm in range(16):
            ph = ps_mm.tile([128, R], f32, tag="mm")
            for dc in range(4):
                nc.tensor.matmul(
                    ph[:],
                    lhsT=W1_s[:, dc, fm * 128:(fm + 1) * 128],
                    rhs=out1T[:, dc, :],
                    start=(dc == 0), stop=(dc == 3),
                )
            nc.scalar.activation(
                h1T[:, fm, :], ph[:], AF.Relu, bias=b1s[:, fm:fm + 1]
            )
        w = out1  # in place: w = out1 + out2
        for qc in range(4):
            po = ps_mm.tile([128, D], f32, tag="mm")
            for fm in range(16):
                nc.tensor.matmul(
                    po[:],
                    lhsT=h1T[:, fm, qc * 128:(qc + 1) * 128],
                    rhs=W2_s[:, fm // 4, fm % 4, :],
                    start=(fm == 0), stop=False,
                )
            nc.tensor.matmul(
                po[:], lhsT=ones_row[:], rhs=b2_r[:], start=False, stop=True
            )
            nc.vector.tensor_tensor(w[:, qc, :], po[:], out1[:, qc, :], ALU.add)

        cout2 = stats_start(w, "b")
        stb2 = stats_finish(cout2, "b")
        for qc in range(4):
            n_t = evac.tile([128, D], f32, tag="evac")
            nc.scalar.activation(
                n_t[:], w[:, qc, :], AF.Identity,
                bias=stb2[:, 1:2], scale=stb2[:, 0:1],
            )
            nc.vector.tensor_tensor(n_t[:], n_t[:], g_nat[:, qc, :], ALU.mult)
            nc.vector.tensor_tensor(n_t[:], n_t[:], b_nat[:, qc, :], ALU.add)
            nc.sync.dma_start(fin_v[:, qc, :], n_t[:])

    split_waits(nc)
    return nc


def _prep(inp):
    """Host-side layout prep: cast weights to bf16 and pre-arrange into the
    exact SBUF layouts the kernel uses. Returns the shared (weight) arrays
    and a per-core function for the row-sharded tensors."""
    f = {k: np.ascontiguousarray(np.asarray(v, dtype=np.float32))
         for k, v in inp.items()}

    def tile128(a):  # [(c 128), n] -> [128, c, n]
        c = a.shape[0] // 128
        return np.ascontiguousarray(
            a.reshape(c, 128, a.shape[1]).transpose(1, 0, 2))

    def pack_heads(w):  # [H, D, E] -> [D, 512] with he = (h//2)*128+(h%2)*64+e
        out = np.zeros((D, D), np.float32)
        for h in range(H):
            out[:, (h // 2) * 128 + (h % 2) * 64:
                (h // 2) * 128 + (h % 2) * 64 + E] = w[h]
        return out

    shared = dict(
        wq=tile128(pack_heads(f["Wq"])).astype(BF16),
        wk=tile128(pack_heads(f["Wk"])).astype(BF16),
        wv=tile128(pack_heads(f["Wv"])).astype(BF16),
        wo_s=np.ascontiguousarray(
            f["Wo"].reshape(H, E, D).transpose(1, 0, 2)).astype(BF16),
        wo_p=np.ascontiguousarray(
            f["Wo"].reshape(4, 2, E, D).transpose(1, 2, 0, 3)
            .reshape(128, 4, D)).astype(BF16),
        w1=tile128(f["W1"]).astype(BF16),
        w2=np.ascontiguousarray(
            f["W2"].reshape(4, 4, 128, D).transpose(2, 0, 1, 3)).astype(BF16),
        bqs2=np.ascontiguousarray(
            f["bq"].reshape(4, 2, E).transpose(1, 2, 0).reshape(128, 4)),
        bks2=np.ascontiguousarray(
            f["bk"].reshape(4, 2, E).transpose(1, 2, 0).reshape(128, 4)),
        bvs2=np.ascontiguousarray(
            f["bv"].reshape(4, 2, E).transpose(1, 2, 0).reshape(128, 4)),
        bv_bc=np.ascontiguousarray(np.tile(f["bv"].reshape(1, D), (128, 1))),
        b1s=np.ascontiguousarray(f["b1"].reshape(16, 128).T),
        bo_r=f["bo"].reshape(1, D).astype(BF16),
        b2_r=f["b2"].reshape(1, D).astype(BF16),
    )

    def per_core(c):
        rows = slice(c * R, (c + 1) * R)
        xr = f["x"][rows]
        return dict(
            xro=tile128(xr),
            xrT=tile128(np.ascontiguousarray(xr.T)).astype(BF16),
            g_nat=tile128(f["ln_g"][rows]).astype(BF16),
            b_nat=tile128(f["ln_b"][rows]).astype(BF16),
            gT=tile128(np.ascontiguousarray(f["ln_g"][rows].T)).astype(BF16),
            bT=tile128(np.ascontiguousarray(f["ln_b"][rows].T)).astype(BF16),
            **shared,
        )

    return [per_core(c) for c in range(N_CORES)]


_NC_CACHE = None


def _get_nc():
    global _NC_CACHE
    if _NC_CACHE is None:
        _NC_CACHE = build_nc()
    return _NC_CACHE


def make_in_maps(inputs):
    return _prep(inputs)


def kernel(**inputs):
    in_maps = _prep(inputs)
    nc = _get_nc()
    res = run_bass_kernel_spmd(nc, in_maps, list(range(N_CORES)))
    final = np.concatenate([res.results[c]["final_rows"] for c in range(N_CORES)])
    Kp = np.concatenate([res.results[c]["Kp_rows"] for c in range(N_CORES)])
    Vp = np.concatenate([res.results[c]["Vp_rows"] for c in range(N_CORES)])
    return (final, Kp, Vp)


# revision 19
# speedup vs baseline: 1.5569x; 1.0624x over previous
"""Trainium2 Bass kernel for nn_Encoder (S=4096, D=512, H=8, E=64).

Sharding: sequence-parallel over 8 cores, but K/V are computed distributed:
each core projects Q/K/V only for its OWN 512 rows, then a single AllGather
(~1 MB/rank) assembles the full K^T and V' on every core. Attention, the
output projection, the global LayerNorms (two 8-byte AllReduces for the
joint [S, D] statistics) and the MLP then run on the own-row shard. The
host concatenates the per-core row shards.

Key differences vs the earlier replicated-KV version:
  - no replicated full-S K/V projection (was ~55us of PE per core)
  - K^T / V' live entirely in SBUF during attention (no DRAM scratch
    streaming; the old version re-read V' four times)
  - weights are cast to bf16 and pre-laid-out on the host, halving weight
    HBM traffic and removing all on-chip transpose/cast preludes
  - all per-head tensors use 64-partition tiles at base 0 (K=64 matmuls)
  - Kp fills the AllGather window; Vp + z-transposes fill the LN1
    AllReduce window.
"""

import os

os.environ.setdefault("JAX_PLATFORMS", "axon")

import numpy as np
import ml_dtypes

import concourse.bass as bass
import concourse.tile as tile
from concourse import mybir
from concourse.bass_utils import run_bass_kernel_spmd
from concourse.masks import make_identity

dt = mybir.dt
AF = mybir.ActivationFunctionType
ALU = mybir.AluOpType
AX = mybir.AxisListType

N_CORES = 8
S, D, H, E = 4096, 512, 8, 64
F = 4 * D          # 2048
R = S // N_CORES   # 512 rows per core
EPS = 1e-5
SCALE = 1.0 / float(np.sqrt(E))
INV_SD = 1.0 / float(S * D)

KT_ELEMS = 128 * 4 * 512          # KTo dump  [p, mc, t]
VP_ELEMS = 128 * 4 * 8 * 65       # VPo dump  [p, tc, h, e']
AG_ELEMS = KT_ELEMS + VP_ELEMS

BF16 = ml_dtypes.bfloat16


def split_waits(nc):
    """Walrus codegen allows only one sync-wait per HW instruction. Move
    extra waits onto single-wait NoOps inserted before, same engine queue."""
    import bass_rust

    n = 0
    for bb in nc.m.functions[0].blocks:
        new_list = []
        changed = False
        for ins in bb.instructions:
            si = ins.sync_info
            if si is not None and si.on_wait is not None and len(si.on_wait) > 1:
                waits = list(si.on_wait)
                for w in waits[:-1]:
                    nop = bass_rust.InstNoOp(name=f"I-xwait-{n}")
                    n += 1
                    nop.engine = ins.engine
                    nop.sync_info = bass_rust.SyncInfo(on_wait=[w], on_update=[])
                    nc.register_instruction(nop)
                    new_list.append(nop)
                si.on_wait = waits[-1:]
                ins.sync_info = si
                changed = True
            new_list.append(ins)
        if changed:
            bb.instructions = new_list
    return nc


def build_nc():
    import contextlib

    nc = bass.Bass("TRN2", debug=False, num_devices=N_CORES)
    f32, f32r, bf16 = dt.float32, dt.float32r, dt.bfloat16

    # ---- I/O (all host-prepped layouts) -------------------------------
    xro_d = nc.dram_tensor("xro", [128, 4, D], f32, kind="ExternalInput").ap()
    xrT_d = nc.dram_tensor("xrT", [128, 4, R], bf16, kind="ExternalInput").ap()
    wq_d = nc.dram_tensor("wq", [128, 4, D], bf16, kind="ExternalInput").ap()
    wk_d = nc.dram_tensor("wk", [128, 4, D], bf16, kind="ExternalInput").ap()
    wv_d = nc.dram_tensor("wv", [128, 4, D], bf16, kind="ExternalInput").ap()
    wos_d = nc.dram_tensor("wo_s", [64, H, D], bf16, kind="ExternalInput").ap()
    wop_d = nc.dram_tensor("wo_p", [128, 4, D], bf16, kind="ExternalInput").ap()
    w1_d = nc.dram_tensor("w1", [128, 4, F], bf16, kind="ExternalInput").ap()
    w2_d = nc.dram_tensor("w2", [128, 4, 4, D], bf16, kind="ExternalInput").ap()
    bqs_d = nc.dram_tensor("bqs2", [128, 4], f32, kind="ExternalInput").ap()
    bks2_d = nc.dram_tensor("bks2", [128, 4], f32, kind="ExternalInput").ap()
    bvs2_d = nc.dram_tensor("bvs2", [128, 4], f32, kind="ExternalInput").ap()
    bvbc_d = nc.dram_tensor("bv_bc", [128, D], f32, kind="ExternalInput").ap()
    b1s_d = nc.dram_tensor("b1s", [128, 16], f32, kind="ExternalInput").ap()
    bor_d = nc.dram_tensor("bo_r", [1, D], bf16, kind="ExternalInput").ap()
    b2r_d = nc.dram_tensor("b2_r", [1, D], bf16, kind="ExternalInput").ap()
    gnat_d = nc.dram_tensor("g_nat", [128, 4, D], bf16, kind="ExternalInput").ap()
    bnat_d = nc.dram_tensor("b_nat", [128, 4, D], bf16, kind="ExternalInput").ap()
    gT_d = nc.dram_tensor("gT", [128, 4, R], bf16, kind="ExternalInput").ap()
    bT_d = nc.dram_tensor("bT", [128, 4, R], bf16, kind="ExternalInput").ap()

    fin_d = nc.dram_tensor("final_rows", [R, D], f32, kind="ExternalOutput").ap()
    kp_d = nc.dram_tensor("Kp_rows", [R, D], f32, kind="ExternalOutput").ap()
    vp_d = nc.dram_tensor("Vp_rows", [R, D], f32, kind="ExternalOutput").ap()

    # row index q = qc*128 + p everywhere
    fin_v = fin_d.rearrange("(c p) d -> p c d", p=128)
    kp_v = kp_d.rearrange("(c p) d -> p c d", p=128)
    vp_v = vp_d.rearrange("(c p) d -> p c d", p=128)

    with tile.TileContext(nc) as tc, contextlib.ExitStack() as ctx, \
            nc.allow_low_precision(reason="bf16 matmul operands, fp32 accumulate"):
        ep = ctx.enter_context

        # ---- pools ----------------------------------------------------
        single = ep(tc.tile_pool(name="single", bufs=1))
        wpool = ep(tc.tile_pool(name="wpool", bufs=1))
        kt_p = ep(tc.tile_pool(name="ktp", bufs=2))      # K^T pair per pass, 1MB
        vp_p = ep(tc.tile_pool(name="vpp", bufs=8))      # V' per rank, 0.53MB
        pexp_p = ep(tc.tile_pool(name="pexp", bufs=3))
        evac = ep(tc.tile_pool(name="evac", bufs=3))
        otr_p = ep(tc.tile_pool(name="otr", bufs=2))
        wk_p = ep(tc.tile_pool(name="wk", bufs=2))
        # psum: tag "mm" 3x2banks + tag "po" 2x1bank = 8 banks
        ps_mm = ep(tc.tile_pool(name="ps_mm", bufs=3, space="PSUM"))
        ps_po = ep(tc.tile_pool(name="ps_po", bufs=2, space="PSUM"))
        dram = ep(tc.tile_pool(name="dram", bufs=1, space="DRAM"))

        # AllGather buffers
        ag_in = dram.tile([AG_ELEMS], bf16)
        ag_out = dram.tile([N_CORES, AG_ELEMS], bf16, addr_space="Shared")
        agi_kt = ag_in[0:KT_ELEMS].rearrange("(p mc t) -> p mc t", p=128, mc=4)
        agi_vp = ag_in[KT_ELEMS:AG_ELEMS].rearrange(
            "(p tc h e) -> p tc h e", p=128, tc=4, h=H)
        # per-head K^T view across ranks: [64e, rank, t]
        ago_kt = ag_out[:, 0:KT_ELEMS].rearrange(
            "r (p mc t) -> p mc r t", p=128, mc=4)  # p = (h%2)*64+e, mc = h//2
        ago_vp = ag_out[:, KT_ELEMS:AG_ELEMS].rearrange(
            "r (p x) -> r p x", p=128)              # x = tc*520 + h*65 + e'

        # ---- constants ------------------------------------------------
        ident = single.tile([128, 128], f32)
        make_identity(nc, ident[:])
        ones1 = single.tile([1, 128], f32)
        nc.vector.memset(ones1[:], 1.0)
        ones_row = single.tile([1, 128], bf16)
        nc.vector.tensor_copy(ones_row[:], ones1[:])
        ones_row_r = single.tile([1, 128], f32r)
        nc.vector.tensor_copy(ones_row_r[:], ones1[:])
        ones8 = single.tile([128, 8], bf16)
        nc.vector.memset(ones8[:], 1.0)
        onesP = single.tile([128, 1], f32)
        nc.vector.memset(onesP[:], 1.0)
        eps_t = single.tile([1, 1], f32)
        nc.vector.memset(eps_t[:], EPS)

        # ---- loads on the AG critical path first ----------------------
        xrT = single.tile([128, 4, R], bf16)      # x^T own rows (host prepped)
        nc.sync.dma_start(xrT[:], xrT_d)
        w_k = wpool.tile([128, 4, D], bf16)
        nc.sync.dma_start(w_k[:], wk_d)
        w_v = wpool.tile([128, 4, D], bf16)
        nc.sync.dma_start(w_v[:], wv_d)
        bks2 = single.tile([128, 4], f32)
        nc.sync.dma_start(bks2[:], bks2_d)
        bvs2 = single.tile([128, 4], f32)
        nc.sync.dma_start(bvs2[:], bvs2_d)
        bv_bc = single.tile([128, D], f32)
        nc.sync.dma_start(bv_bc[:], bvbc_d)

        # ---- phase 1: own-row projections -----------------------------
        def own_proj_packed(dst, w_t, bias2_t):
            """dst[128, mc, R] = pair-packed (x_rows @ W)^T + b, p=(h%2)*64+e."""
            for mc in range(4):
                pq = ps_mm.tile([128, R], f32, tag="mm")
                for dc in range(4):
                    nc.tensor.matmul(
                        pq[:],
                        lhsT=w_t[:, dc, mc * 128:(mc + 1) * 128],
                        rhs=xrT[:, dc, :],
                        start=(dc == 0), stop=(dc == 3),
                    )
                nc.vector.tensor_scalar_add(
                    dst[:, mc, :], pq[:], bias2_t[:, mc:mc + 1]
                )

        KTo = single.tile([128, 4, R], bf16)
        own_proj_packed(KTo, w_k, bks2)
        nc.sync.dma_start(agi_kt, KTo[:])

        VPo = single.tile([128, 4, H, E + 1], bf16)   # [t%128, tc, h, e']
        for vc in range(4):
            pv = ps_mm.tile([128, D], f32, tag="mm")
            for dc in range(4):
                nc.tensor.matmul(
                    pv[:],
                    lhsT=xrT[:, dc, vc * 128:(vc + 1) * 128],
                    rhs=w_v[:, dc, :],
                    start=(dc == 0), stop=(dc == 3),
                )
            nc.vector.tensor_tensor(
                VPo[:, vc, :, 0:E],
                pv[:].rearrange("p (h e) -> p h e", e=E),
                bv_bc[:].rearrange("p (h e) -> p h e", e=E),
                ALU.add,
            )
            nc.vector.tensor_copy(VPo[:, vc, :, E], ones8[:])
        nc.sync.dma_start(agi_vp, VPo[:])

        # ---- AllGather of K^T + V' (1.03 MB per rank) ----------------
        nc.gpsimd.collective_compute(
            "AllGather", ALU.bypass,
            replica_groups=[list(range(N_CORES))],
            ins=[ag_in[:]], outs=[ag_out[:]],
        )

        # ---- AG window: remaining loads, Q^T, V^T packed, Kp ---------
        w_q = wpool.tile([128, 4, D], bf16)
        nc.sync.dma_start(w_q[:], wq_d)
        bqs2 = single.tile([128, 4], f32)
        nc.sync.dma_start(bqs2[:], bqs_d)
        xro = single.tile([128, 4, D], f32)       # x own rows; becomes z
        nc.sync.dma_start(xro[:], xro_d)
        Wo_p = wpool.tile([128, 4, D], bf16)
        nc.sync.dma_start(Wo_p[:], wop_d)
        bo_r = single.tile([1, D], bf16)
        nc.sync.dma_start(bo_r[:], bor_d)
        b2_r = single.tile([1, D], bf16)
        nc.sync.dma_start(b2_r[:], b2r_d)
        b1s = single.tile([128, 16], f32)
        nc.sync.dma_start(b1s[:], b1s_d)

        # QT[:, h, :] holds Q_h^T at rows (h%2)*64..+64, zeros elsewhere, so
        # a pair-packed K^T block serves as shared lhsT for both heads.
        QT = single.tile([128, H, R], bf16)
        nc.vector.memset(QT[:], 0.0)
        for c in range(4):
            pq = ps_mm.tile([128, R], f32, tag="mm")
            for dc in range(4):
                nc.tensor.matmul(
                    pq[0:64, :],
                    lhsT=w_q[:, dc, c * 128:c * 128 + 64],
                    rhs=xrT[:, dc, :],
                    start=(dc == 0), stop=(dc == 3),
                )
            for dc in range(4):
                nc.tensor.matmul(
                    pq[64:128, :],
                    lhsT=w_q[:, dc, c * 128 + 64:c * 128 + 128],
                    rhs=xrT[:, dc, :],
                    start=(dc == 0), stop=(dc == 3),
                )
            nc.vector.tensor_scalar_add(
                QT[0:64, 2 * c, :], pq[0:64, :], bqs2[0:64, c:c + 1]
            )
            nc.vector.tensor_scalar_add(
                QT[64:128, 2 * c + 1, :], pq[64:128, :], bqs2[64:128, c:c + 1]
            )

        VTo = single.tile([128, 4, R], bf16)
        own_proj_packed(VTo, w_v, bvs2)

        def wo_project_packed(src_T, out_view):
            """out_view rows = concat_h(src) @ Wo + bo (src packed [128,4,R])."""
            for qc in range(4):
                po = ps_mm.tile([128, D], f32, tag="mm")
                for mc in range(4):
                    nc.tensor.matmul(
                        po[:],
                        lhsT=src_T[:, mc, qc * 128:(qc + 1) * 128],
                        rhs=Wo_p[:, mc, :],
                        start=(mc == 0), stop=False,
                    )
                nc.tensor.matmul(
                    po[:], lhsT=ones_row[:], rhs=bo_r[:], start=False, stop=True
                )
                ot = evac.tile([128, D], f32, tag="evac")
                nc.vector.tensor_copy(ot[:], po[:])
                nc.sync.dma_start(out_view[:, qc, :], ot[:])

        wo_project_packed(KTo, kp_v)    # Kp during the AG latency window

        # ---- attention readbacks -------------------------------------
        vp_r = []
        for r in range(N_CORES):
            t = vp_p.tile([128, 4, H, E + 1], bf16, tag="vpr", name=f"vpr{r}")
            nc.sync.dma_start(
                t[:].rearrange("p tc h e -> p (tc h e)"), ago_vp[r]
            )
            vp_r.append(t)

        # ---- phase 2: attention (4 passes x 2 heads) ------------------
        OT = single.tile([64, H, R], bf16)  # normalized outH^T
        w1_loaded = False
        W1_s = wpool.tile([128, 4, F], bf16)
        W2_s = wpool.tile([128, 4, 4, D], bf16)

        for pass_ in range(4):
            h0, h1 = 2 * pass_, 2 * pass_ + 1
            # pair-packed K^T for both heads: p = (h%2)*64 + e
            kt_t = kt_p.tile([128, N_CORES, R], bf16, tag="kt", name=f"kt{pass_}")
            nc.sync.dma_start(kt_t[:], ago_kt[:, pass_, :, :])
            po_a = ps_po.tile([E + 1, R], f32, tag="po")
            po_b = ps_po.tile([E + 1, R], f32, tag="po")
            # AV runs one chunk behind the logits/exp stream so the in-order
            # PE queue never stalls on the current chunk's exp.
            pend = None
            for ch in range(32):
                r, tc = ch // 4, ch % 4
                pl = ps_mm.tile([128, 2, R], f32, tag="mm")
                nc.tensor.matmul(
                    pl[:, 0, :],
                    lhsT=kt_t[:, r, tc * 128:(tc + 1) * 128],
                    rhs=QT[:, h0, :], start=True, stop=True,
                )
                nc.tensor.matmul(
                    pl[:, 1, :],
                    lhsT=kt_t[:, r, tc * 128:(tc + 1) * 128],
                    rhs=QT[:, h1, :], start=True, stop=True,
                )
                pexp = pexp_p.tile([128, 2, R], bf16, tag="pexp")
                nc.scalar.activation(pexp[:], pl[:], AF.Exp, scale=SCALE)
                if pend is not None:
                    pr_, ptc, ppexp, pch = pend
                    nc.tensor.matmul(
                        po_a[:], lhsT=vp_r[pr_][:, ptc, h0, :],
                        rhs=ppexp[:, 0, :], start=(pch == 0), stop=False,
                    )
                    nc.tensor.matmul(
                        po_b[:], lhsT=vp_r[pr_][:, ptc, h1, :],
                        rhs=ppexp[:, 1, :], start=(pch == 0), stop=False,
                    )
                pend = (r, tc, pexp, ch)
            pr_, ptc, ppexp, pch = pend
            nc.tensor.matmul(
                po_a[:], lhsT=vp_r[pr_][:, ptc, h0, :], rhs=ppexp[:, 0, :],
                start=False, stop=True,
            )
            nc.tensor.matmul(
                po_b[:], lhsT=vp_r[pr_][:, ptc, h1, :], rhs=ppexp[:, 1, :],
                start=False, stop=True,
            )
            # normalize rows 0..63 by the ones-column row 64
            for po_t, h in ((po_a, h0), (po_b, h1)):
                otr = otr_p.tile([E + 1, R], f32, tag="otr")
                nc.vector.tensor_copy(otr[:], po_t[:])
                rden = otr_p.tile([1, R], f32r, tag="rden")
                nc.vector.reciprocal(rden[:], otr[E:E + 1, :])
                pb = ps_po.tile([E, R], f32, tag="po")
                nc.tensor.matmul(
                    pb[:], lhsT=ones_row_r[:, 0:E], rhs=rden[:],
                    start=True, stop=True,
                )
                nc.vector.tensor_tensor(OT[:, h, :], otr[0:E, :], pb[:], ALU.mult)
            if pass_ == 1 and not w1_loaded:
                # stream the MLP weights while attention still runs
                w1_loaded = True
                nc.sync.dma_start(W1_s[:], w1_d)
                nc.sync.dma_start(W2_s[:], w2_d)

        # ---- phase 3: out proj + residual + global LN1 ----------------
        # Wo_s reuses a freed K^T slot (same pool tag, disjoint lifetime)
        Wo_s = kt_p.tile([64, H, D], bf16, tag="kt", name="Wo_s")
        nc.sync.dma_start(Wo_s[:], wos_d)
        z = xro  # in place: z = x + out
        for qc in range(4):
            po = ps_mm.tile([128, D], f32, tag="mm")
            for h in range(H):
                nc.tensor.matmul(
                    po[:],
                    lhsT=OT[:, h, qc * 128:(qc + 1) * 128],
                    rhs=Wo_s[:, h, :],
                    start=(h == 0), stop=False,
                )
            nc.tensor.matmul(
                po[:], lhsT=ones_row[:], rhs=bo_r[:], start=False, stop=True
            )
            nc.vector.tensor_tensor(z[:, qc, :], po[:], xro[:, qc, :], ALU.add)

        def stats_start(src_t, tag):
            """Partial [sum, sumsq] -> AllGather (cheaper floor than
            AllReduce for 8 B); returns the gathered [8, 2] dram tile."""
            sums = wk_p.tile([128, 5], f32, tag=f"sums{tag}")
            nc.vector.tensor_reduce(
                out=sums[:, 0:1], in_=src_t[:], axis=AX.XY, op=ALU.add
            )
            for qc in range(4):
                sqv = evac.tile([128, D], f32, tag="evac")
                nc.scalar.activation(
                    sqv[:], src_t[:, qc, :], AF.Square,
                    accum_out=sums[:, 1 + qc:2 + qc],
                )
            pr = ps_po.tile([1, 5], f32, tag="po")
            nc.tensor.matmul(
                pr[:], lhsT=onesP[:], rhs=sums[:], start=True, stop=True
            )
            part = wk_p.tile([1, 2], f32, tag=f"part{tag}")
            nc.vector.tensor_copy(part[:, 0:1], pr[:, 0:1])
            nc.vector.tensor_reduce(
                out=part[:, 1:2], in_=pr[:, 1:5], axis=AX.X, op=ALU.add
            )
            cin = dram.tile([1, 2], f32)
            cout = dram.tile([N_CORES, 2], f32)
            nc.sync.dma_start(cin[:], part[:])
            nc.gpsimd.collective_compute(
                "AllGather", ALU.bypass,
                replica_groups=[list(range(N_CORES))],
                ins=[cin[:]], outs=[cout[:]],
            )
            return cout

        def stats_finish(cout, tag):
            """-> [128, 2] sbuf tile: [:,0]=rstd, [:,1]=-mu*rstd (global)."""
            tot8 = wk_p.tile([N_CORES, 2], f32, tag=f"tot8{tag}")
            nc.sync.dma_start(tot8[:], cout[:])
            pr8 = ps_po.tile([1, 2], f32, tag="po")
            nc.tensor.matmul(
                pr8[:], lhsT=onesP[0:N_CORES, :], rhs=tot8[:],
                start=True, stop=True,
            )
            tot = wk_p.tile([1, 2], f32, tag=f"tot{tag}")
            nc.vector.tensor_copy(tot[:], pr8[:])
            sc = wk_p.tile([1, 6], f32, tag=f"sc{tag}")
            mu, m2 = sc[0:1, 0:1], sc[0:1, 1:2]
            nc.vector.tensor_scalar_mul(mu, tot[0:1, 0:1], INV_SD)
            nc.vector.tensor_scalar_mul(m2, tot[0:1, 1:2], INV_SD)
            nc.vector.tensor_tensor(sc[0:1, 2:3], mu, mu, ALU.mult)
            nc.vector.tensor_tensor(sc[0:1, 3:4], m2, sc[0:1, 2:3], ALU.subtract)
            nc.scalar.activation(sc[0:1, 4:5], sc[0:1, 3:4], AF.Sqrt, bias=eps_t[:])
            st2 = wk_p.tile([1, 2], f32r, tag=f"st2{tag}")
            nc.vector.reciprocal(st2[0:1, 0:1], sc[0:1, 4:5])        # rstd
            nc.vector.tensor_tensor(sc[0:1, 5:6], mu, st2[0:1, 0:1], ALU.mult)
            nc.vector.tensor_scalar_mul(st2[0:1, 1:2], sc[0:1, 5:6], -1.0)
            pbc = ps_po.tile([128, 2], f32, tag="po")
            nc.tensor.matmul(pbc[:], lhsT=ones_row_r[:], rhs=st2[:],
                             start=True, stop=True)
            stb = wk_p.tile([128, 2], f32, tag=f"stb{tag}")
            nc.vector.tensor_copy(stb[:], pbc[:])
            return stb

        cout1 = stats_start(z, "a")
        # AR1 latency window: Vp projection + z^T transposes
        wo_project_packed(VTo, vp_v)
        zT = single.tile([128, 4, R], bf16)        # z^T for the MLP path
        for dc in range(4):
            for qc in range(4):
                ptr = ps_po.tile([128, 128], f32, tag="po")
                nc.tensor.transpose(
                    ptr[:], z[:, qc, dc * 128:(dc + 1) * 128], ident[:]
                )
                nc.vector.tensor_copy(zT[:, dc, qc * 128:(qc + 1) * 128], ptr[:])
        stb1 = stats_finish(cout1, "a")

        # LN1 in both orientations (natural for residual, ^T for the MLP)
        g_nat = single.tile([128, 4, D], bf16)
        nc.sync.dma_start(g_nat[:], gnat_d)
        b_nat = single.tile([128, 4, D], bf16)
        nc.sync.dma_start(b_nat[:], bnat_d)


        out1 = single.tile([128, 4, D], f32)
        for qc in range(4):
            n_t = evac.tile([128, D], f32, tag="evac")
            nc.scalar.activation(
                n_t[:], z[:, qc, :], AF.Identity,
                bias=stb1[:, 1:2], scale=stb1[:, 0:1],
            )
            nc.vector.tensor_tensor(n_t[:], n_t[:], g_nat[:, qc, :], ALU.mult)
            nc.vector.tensor_tensor(out1[:, qc, :], n_t[:], b_nat[:, qc, :], ALU.add)
        out1T = single.tile([128, 4, R], bf16)
        for dc in range(4):
            gT_t = evac.tile([128, R], bf16, tag="evacT")
            nc.sync.dma_start(gT_t[:], gT_d[:, dc, :])
            bT_t = evac.tile([128, R], bf16, tag="evacT")
            nc.sync.dma_start(bT_t[:], bT_d[:, dc, :])
            nT = evac.tile([128, R], bf16, tag="evacT")
            nc.scalar.activation(
                nT[:], zT[:, dc, :], AF.Identity,
                bias=stb1[:, 1:2], scale=stb1[:, 0:1],
            )
            nc.vector.tensor_tensor(nT[:], nT[:], gT_t[:], ALU.mult)
            nc.vector.tensor_tensor(out1T[:, dc, :], nT[:], bT_t[:], ALU.add)

        # ---- phase 4: MLP + residual + global LN2 --------------------
        h1T = single.tile([128, 16, R], bf16)
        for fm in range(16):
            ph = ps_mm.tile([128, R], f32, tag="mm")
            for dc in range(4):
                nc.tensor.matmul(
                    ph[:],
                    lhsT=W1_s[:, dc, fm * 128:(fm + 1) * 128],
                    rhs=out1T[:, dc, :],
                    start=(dc == 0), stop=(dc == 3),
                )
            nc.scalar.activation(
                h1T[:, fm, :], ph[:], AF.Relu, bias=b1s[:, fm:fm + 1]
            )
        w = out1  # in place: w = out1 + out2
        for qc in range(4):
            po = ps_mm.tile([128, D], f32, tag="mm")
            for fm in range(16):
                nc.tensor.matmul(
                    po[:],
                    lhsT=h1T[:, fm, qc * 128:(qc + 1) * 128],
                    rhs=W2_s[:, fm // 4, fm % 4, :],
                    start=(fm == 0), stop=False,
                )
            nc.tensor.matmul(
                po[:], lhsT=ones_row[:], rhs=b2_r[:], start=False, stop=True
            )
            nc.vector.tensor_tensor(w[:, qc, :], po[:], out1[:, qc, :], ALU.add)

        cout2 = stats_start(w, "b")
        stb2 = stats_finish(cout2, "b")
        for qc in range(4):
            n_t = evac.tile([128, D], f32, tag="evac")
            nc.scalar.activation(
                n_t[:], w[:, qc, :], AF.Identity,
                bias=stb2[:, 1:2], scale=stb2[:, 0:1],
            )
            nc.vector.tensor_tensor(n_t[:], n_t[:], g_nat[:, qc, :], ALU.mult)
            nc.vector.tensor_tensor(n_t[:], n_t[:], b_nat[:, qc, :], ALU.add)
            nc.sync.dma_start(fin_v[:, qc, :], n_t[:])

    split_waits(nc)
    return nc


def _prep(inp):
    """Host-side layout prep: cast weights to bf16 and pre-arrange into the
    exact SBUF layouts the kernel uses. Returns the shared (weight) arrays
    and a per-core function for the row-sharded tensors."""
    f = {k: np.ascontiguousarray(np.asarray(v, dtype=np.float32))
         for k, v in inp.items()}

    def tile128(a):  # [(c 128), n] -> [128, c, n]
        c = a.shape[0] // 128
        return np.ascontiguousarray(
            a.reshape(c, 128, a.shape[1]).transpose(1, 0, 2))

    def pack_heads(w):  # [H, D, E] -> [D, 512] with he = (h//2)*128+(h%2)*64+e
        out = np.zeros((D, D), np.float32)
        for h in range(H):
            out[:, (h // 2) * 128 + (h % 2) * 64:
                (h // 2) * 128 + (h % 2) * 64 + E] = w[h]
        return out

    shared = dict(
        wq=tile128(pack_heads(f["Wq"])).astype(BF16),
        wk=tile128(pack_heads(f["Wk"])).astype(BF16),
        wv=tile128(pack_heads(f["Wv"])).astype(BF16),
        wo_s=np.ascontiguousarray(
            f["Wo"].reshape(H, E, D).transpose(1, 0, 2)).astype(BF16),
        wo_p=np.ascontiguousarray(
            f["Wo"].reshape(4, 2, E, D).transpose(1, 2, 0, 3)
            .reshape(128, 4, D)).astype(BF16),
        w1=tile128(f["W1"]).astype(BF16),
        w2=np.ascontiguousarray(
            f["W2"].reshape(4, 4, 128, D).transpose(2, 0, 1, 3)).astype(BF16),
        bqs2=np.ascontiguousarray(
            f["bq"].reshape(4, 2, E).transpose(1, 2, 0).reshape(128, 4)),
        bks2=np.ascontiguousarray(
            f["bk"].reshape(4, 2, E).transpose(1, 2, 0).reshape(128, 4)),
        bvs2=np.ascontiguousarray(
            f["bv"].reshape(4, 2, E).transpose(1, 2, 0).reshape(128, 4)),
        bv_bc=np.ascontiguousarray(np.tile(f["bv"].reshape(1, D), (128, 1))),
        b1s=np.ascontiguousarray(f["b1"].reshape(16, 128).T),
        bo_r=f["bo"].reshape(1, D).astype(BF16),
        b2_r=f["b2"].reshape(1, D).astype(BF16),
    )

    def per_core(c):
        rows = slice(c * R, (c + 1) * R)
        xr = f["x"][rows]
        return dict(
            xro=tile128(xr),
            xrT=tile128(np.ascontiguousarray(xr.T)).astype(BF16),
            g_nat=tile128(f["ln_g"][rows]).astype(BF16),
            b_nat=tile128(f["ln_b"][rows]).astype(BF16),
            gT=tile128(np.ascontiguousarray(f["ln_g"][rows].T)).astype(BF16),
            bT=tile128(np.ascontiguousarray(f["ln_b"][rows].T)).astype(BF16),
            **shared,
        )

    return [per_core(c) for c in range(N_CORES)]


_NC_CACHE = None


def _get_nc():
    global _NC_CACHE
    if _NC_CACHE is None:
        _NC_CACHE = build_nc()
    return _NC_CACHE


def make_in_maps(inputs):
    return _prep(inputs)


def kernel(**inputs):
    in_maps = _prep(inputs)
    nc = _get_nc()
    res = run_bass_kernel_spmd(nc, in_maps, list(range(N_CORES)))
    final = np.concatenate([res.results[c]["final_rows"] for c in range(N_CORES)])
    Kp = np.concatenate([res.results[c]["Kp_rows"] for c in range(N_CORES)])
    Vp = np.concatenate([res.results[c]["Vp_rows"] for c in range(N_CORES)])
    return (final, Kp, Vp)


# revision 20
# speedup vs baseline: 1.6034x; 1.0299x over previous
"""Trainium2 Bass kernel for nn_Encoder (S=4096, D=512, H=8, E=64).

Sharding: sequence-parallel over 8 cores with distributed K/V: each core
projects Q/K/V only for its OWN 512 rows, then four pipelined AllGather
slices (~0.26 MB/rank each) assemble the full K^T and V' on every core
while attention is already consuming the earlier slices. Attention, the
output projection, the global LayerNorms (two 8-byte AllGathers for the
joint [S, D] statistics) and the MLP run on the own-row shard. The host
concatenates the per-core row shards.

Scheduling structure:
  - a dummy 4-byte AllGather is issued first so the one-time collective
    rendezvous barrier + first-collective setup cost runs during the
    startup loads instead of before the K/V gather
  - slice j carries K^T for head-pair j plus V' for key-chunk column j;
    the attention pass loop iterates key chunks column-outer so slice j
    is consumed j-th
  - AV matmuls run one chunk behind the logits/exp stream so the
    in-order PE queue never stalls on the current chunk's exp
  - Kp/Vp (the K/V side outputs) fill the AllGather latency window
  - when ln_g == 1 and ln_b == 0 (the reference initializer), LN1's
    affine commutes into the MLP first matmul: z^T @ W1 is computed
    during the LN1-stats collective and h1 = relu(rstd*(z^T@W1) +
    (b1 - mu*rstd*colsum(W1))) needs only one activation per f-tile.
"""

import os

os.environ.setdefault("JAX_PLATFORMS", "axon")

import numpy as np
import ml_dtypes

import concourse.bass as bass
import concourse.tile as tile
from concourse import mybir
from concourse.bass_utils import run_bass_kernel_spmd
from concourse.masks import make_identity

dt = mybir.dt
AF = mybir.ActivationFunctionType
ALU = mybir.AluOpType
AX = mybir.AxisListType

N_CORES = 8
S, D, H, E = 4096, 512, 8, 64
F = 4 * D          # 2048
R = S // N_CORES   # 512 rows per core
EPS = 1e-5
SCALE = 1.0 / float(np.sqrt(E))
INV_SD = 1.0 / float(S * D)

SL_KT = 128 * 512            # K^T slice dump [p, t]
SL_VP = 128 * 8 * 65         # V' slice dump [p, h, e']
SL_ELEMS = SL_KT + SL_VP     # one AG slice, per rank

BF16 = ml_dtypes.bfloat16


def split_waits(nc):
    """Walrus codegen allows only one sync-wait per HW instruction. Move
    extra waits onto single-wait NoOps inserted before, same engine queue."""
    import bass_rust

    n = 0
    for bb in nc.m.functions[0].blocks:
        new_list = []
        changed = False
        for ins in bb.instructions:
            si = ins.sync_info
            if si is not None and si.on_wait is not None and len(si.on_wait) > 1:
                waits = list(si.on_wait)
                for w in waits[:-1]:
                    nop = bass_rust.InstNoOp(name=f"I-xwait-{n}")
                    n += 1
                    nop.engine = ins.engine
                    nop.sync_info = bass_rust.SyncInfo(on_wait=[w], on_update=[])
                    nc.register_instruction(nop)
                    new_list.append(nop)
                si.on_wait = waits[-1:]
                ins.sync_info = si
                changed = True
            new_list.append(ins)
        if changed:
            bb.instructions = new_list
    return nc


def build_nc(fast_ln=True):
    import contextlib

    nc = bass.Bass("TRN2", debug=False, num_devices=N_CORES)
    f32, f32r, bf16 = dt.float32, dt.float32r, dt.bfloat16

    # ---- I/O (all host-prepped layouts) -------------------------------
    xro_d = nc.dram_tensor("xro", [128, 4, D], f32, kind="ExternalInput").ap()
    xrT_d = nc.dram_tensor("xrT", [128, 4, R], bf16, kind="ExternalInput").ap()
    wq_d = nc.dram_tensor("wq", [128, 4, D], bf16, kind="ExternalInput").ap()
    wk_d = nc.dram_tensor("wk", [128, 4, D], bf16, kind="ExternalInput").ap()
    wv_d = nc.dram_tensor("wv", [128, 4, D], bf16, kind="ExternalInput").ap()
    wos_d = nc.dram_tensor("wo_s", [64, H, D], bf16, kind="ExternalInput").ap()
    wop_d = nc.dram_tensor("wo_p", [128, 4, D], bf16, kind="ExternalInput").ap()
    w1_d = nc.dram_tensor("w1", [128, 4, F], bf16, kind="ExternalInput").ap()
    w2_d = nc.dram_tensor("w2", [128, 4, 4, D], bf16, kind="ExternalInput").ap()
    cs1_d = nc.dram_tensor("colsum_w1", [128, 16], f32, kind="ExternalInput").ap()
    bqs_d = nc.dram_tensor("bqs2", [128, 4], f32, kind="ExternalInput").ap()
    bks2_d = nc.dram_tensor("bks2", [128, 4], f32, kind="ExternalInput").ap()
    bvs2_d = nc.dram_tensor("bvs2", [128, 4], f32, kind="ExternalInput").ap()
    bvbc_d = nc.dram_tensor("bv_bc", [128, D], f32, kind="ExternalInput").ap()
    b1s_d = nc.dram_tensor("b1s", [128, 16], f32, kind="ExternalInput").ap()
    bor_d = nc.dram_tensor("bo_r", [1, D], bf16, kind="ExternalInput").ap()
    b2r_d = nc.dram_tensor("b2_r", [1, D], bf16, kind="ExternalInput").ap()
    if not fast_ln:
        gnat_d = nc.dram_tensor("g_nat", [128, 4, D], bf16, kind="ExternalInput").ap()
        bnat_d = nc.dram_tensor("b_nat", [128, 4, D], bf16, kind="ExternalInput").ap()
        gT_d = nc.dram_tensor("gT", [128, 4, R], bf16, kind="ExternalInput").ap()
        bT_d = nc.dram_tensor("bT", [128, 4, R], bf16, kind="ExternalInput").ap()

    fin_d = nc.dram_tensor("final_rows", [R, D], f32, kind="ExternalOutput").ap()
    kp_d = nc.dram_tensor("Kp_rows", [R, D], f32, kind="ExternalOutput").ap()
    vp_d = nc.dram_tensor("Vp_rows", [R, D], f32, kind="ExternalOutput").ap()

    # row index q = qc*128 + p everywhere
    fin_v = fin_d.rearrange("(c p) d -> p c d", p=128)
    kp_v = kp_d.rearrange("(c p) d -> p c d", p=128)
    vp_v = vp_d.rearrange("(c p) d -> p c d", p=128)

    with tile.TileContext(nc) as tc, contextlib.ExitStack() as ctx, \
            nc.allow_low_precision(reason="bf16 matmul operands, fp32 accumulate"):
        ep = ctx.enter_context

        # ---- pools ----------------------------------------------------
        single = ep(tc.tile_pool(name="single", bufs=1))
        wpool = ep(tc.tile_pool(name="wpool", bufs=1))
        kt_p = ep(tc.tile_pool(name="ktp", bufs=2))      # K^T pair per pass
        vp_p = ep(tc.tile_pool(name="vpp", bufs=4))      # V' per tc slice
        pexp_p = ep(tc.tile_pool(name="pexp", bufs=3))
        evac = ep(tc.tile_pool(name="evac", bufs=3))
        otr_p = ep(tc.tile_pool(name="otr", bufs=2))
        wk_p = ep(tc.tile_pool(name="wk", bufs=2))
        # psum: tag "mm" 3x2banks + tag "po" 2x1bank = 8 banks
        ps_mm = ep(tc.tile_pool(name="ps_mm", bufs=3, space="PSUM"))
        ps_po = ep(tc.tile_pool(name="ps_po", bufs=2, space="PSUM"))
        dram = ep(tc.tile_pool(name="dram", bufs=1, space="DRAM"))

        # ---- dummy first collective: absorb the rendezvous barrier ----
        dumb = single.tile([1, 1], f32)
        nc.vector.memset(dumb[:], 0.0)
        dumb_in = dram.tile([1, 1], f32)
        dumb_out = dram.tile([N_CORES, 1], f32, addr_space="Shared")
        nc.sync.dma_start(dumb_in[:], dumb[:])
        nc.gpsimd.collective_compute(
            "AllGather", ALU.bypass,
            replica_groups=[list(range(N_CORES))],
            ins=[dumb_in[:]], outs=[dumb_out[:]],
        )

        # AllGather slice buffers: slice j = K^T pair j + V' chunk-col j
        ag_in, ag_out, agi, ago_kt, ago_vp = [], [], [], [], []
        for j in range(4):
            ai = dram.tile([SL_ELEMS], bf16, name=f"ag_in{j}")
            ao = dram.tile([N_CORES, SL_ELEMS], bf16, addr_space="Shared",
                           name=f"ag_out{j}")
            ag_in.append(ai)
            ag_out.append(ao)
            agi.append(ai[:].rearrange("(p x) -> p x", p=128))
            av = ao[:].rearrange("r (p x) -> p r x", p=128)
            ago_kt.append(av[:, :, 0:512])
            ago_vp.append(av[:, :, 512:512 + 8 * 65])

        # ---- constants ------------------------------------------------
        ident = single.tile([128, 128], f32)
        make_identity(nc, ident[:])
        ones1 = single.tile([1, 128], f32)
        nc.vector.memset(ones1[:], 1.0)
        ones_row = single.tile([1, 128], bf16)
        nc.vector.tensor_copy(ones_row[:], ones1[:])
        ones_row_r = single.tile([1, 128], f32r)
        nc.vector.tensor_copy(ones_row_r[:], ones1[:])
        ones8 = single.tile([128, 8], bf16)
        nc.vector.memset(ones8[:], 1.0)
        onesP = single.tile([128, 1], f32)
        nc.vector.memset(onesP[:], 1.0)
        eps_t = single.tile([1, 1], f32)
        nc.vector.memset(eps_t[:], EPS)

        # ---- loads on the AG critical path first ----------------------
        xrT = single.tile([128, 4, R], bf16)      # x^T own rows (host prepped)
        nc.sync.dma_start(xrT[:], xrT_d)
        w_k = wpool.tile([128, 4, D], bf16)
        nc.sync.dma_start(w_k[:], wk_d)
        w_v = wpool.tile([128, 4, D], bf16)
        nc.sync.dma_start(w_v[:], wv_d)
        bks2 = single.tile([128, 4], f32)
        nc.sync.dma_start(bks2[:], bks2_d)
        bvs2 = single.tile([128, 4], f32)
        nc.sync.dma_start(bvs2[:], bvs2_d)
        bv_bc = single.tile([128, D], f32)
        nc.sync.dma_start(bv_bc[:], bvbc_d)

        # ---- phase 1: own-row K/V projections, sliced + gathered ------
        KTo = single.tile([128, 4, R], bf16)      # kept for Kp
        VPo = single.tile([128, 4, H, E + 1], bf16)
        for j in range(4):
            pq = ps_mm.tile([128, R], f32, tag="mm")
            for dc in range(4):
                nc.tensor.matmul(
                    pq[:],
                    lhsT=w_k[:, dc, j * 128:(j + 1) * 128],
                    rhs=xrT[:, dc, :],
                    start=(dc == 0), stop=(dc == 3),
                )
            nc.vector.tensor_scalar_add(KTo[:, j, :], pq[:], bks2[:, j:j + 1])
            pv = ps_mm.tile([128, D], f32, tag="mm")
            for dc in range(4):
                nc.tensor.matmul(
                    pv[:],
                    lhsT=xrT[:, dc, j * 128:(j + 1) * 128],
                    rhs=w_v[:, dc, :],
                    start=(dc == 0), stop=(dc == 3),
                )
            nc.vector.tensor_tensor(
                VPo[:, j, :, 0:E],
                pv[:].rearrange("p (h e) -> p h e", e=E),
                bv_bc[:].rearrange("p (h e) -> p h e", e=E),
                ALU.add,
            )
            nc.vector.tensor_copy(VPo[:, j, :, E], ones8[:])
            nc.sync.dma_start(
                agi[j][:, 0:512], KTo[:, j, :]
            )
            nc.sync.dma_start(
                agi[j][:, 512:512 + 8 * 65].rearrange("p (h e) -> p h e", e=E + 1),
                VPo[:, j, :, :],
            )
            nc.gpsimd.collective_compute(
                "AllGather", ALU.bypass,
                replica_groups=[list(range(N_CORES))],
                ins=[ag_in[j][:]], outs=[ag_out[j][:]],
            )

        # ---- AG window: remaining loads, Q^T, V^T packed, Kp, Vp ------
        w_q = wpool.tile([128, 4, D], bf16)
        nc.sync.dma_start(w_q[:], wq_d)
        bqs2 = single.tile([128, 4], f32)
        nc.sync.dma_start(bqs2[:], bqs_d)
        Wo_p = wpool.tile([128, 4, D], bf16)
        nc.sync.dma_start(Wo_p[:], wop_d)
        bo_r = single.tile([1, D], bf16)
        nc.sync.dma_start(bo_r[:], bor_d)
        b2_r = single.tile([1, D], bf16)
        nc.sync.dma_start(b2_r[:], b2r_d)
        b1s = single.tile([128, 16], f32)
        nc.sync.dma_start(b1s[:], b1s_d)
        cs1 = single.tile([128, 16], f32)
        nc.sync.dma_start(cs1[:], cs1_d)

        # QT[:, h, :] holds Q_h^T at rows (h%2)*64..+64, zeros elsewhere, so
        # a pair-packed K^T block serves as shared lhsT for both heads.
        QT = single.tile([128, H, R], bf16)
        nc.vector.memset(QT[:], 0.0)
        for c in range(4):
            pq = ps_mm.tile([128, R], f32, tag="mm")
            for dc in range(4):
                nc.tensor.matmul(
                    pq[0:64, :],
                    lhsT=w_q[:, dc, c * 128:c * 128 + 64],
                    rhs=xrT[:, dc, :],
                    start=(dc == 0), stop=(dc == 3),
                )
            for dc in range(4):
                nc.tensor.matmul(
                    pq[64:128, :],
                    lhsT=w_q[:, dc, c * 128 + 64:c * 128 + 128],
                    rhs=xrT[:, dc, :],
                    start=(dc == 0), stop=(dc == 3),
                )
            nc.vector.tensor_scalar_add(
                QT[0:64, 2 * c, :], pq[0:64, :], bqs2[0:64, c:c + 1]
            )
            nc.vector.tensor_scalar_add(
                QT[64:128, 2 * c + 1, :], pq[64:128, :], bqs2[64:128, c:c + 1]
            )

        def own_proj_packed(dst, w_t, bias2_t):
            """dst[128, mc, R] = pair-packed (x_rows @ W)^T + b, p=(h%2)*64+e."""
            for mc in range(4):
                pq = ps_mm.tile([128, R], f32, tag="mm")
                for dc in range(4):
                    nc.tensor.matmul(
                        pq[:],
                        lhsT=w_t[:, dc, mc * 128:(mc + 1) * 128],
                        rhs=xrT[:, dc, :],
                        start=(dc == 0), stop=(dc == 3),
                    )
                nc.vector.tensor_scalar_add(
                    dst[:, mc, :], pq[:], bias2_t[:, mc:mc + 1]
                )

        VTo = single.tile([128, 4, R], bf16)
        own_proj_packed(VTo, w_v, bvs2)

        def wo_project_packed(src_T, out_view):
            """out_view rows = concat_h(src) @ Wo + bo (src packed [128,4,R])."""
            for qc in range(4):
                po = ps_mm.tile([128, D], f32, tag="mm")
                for mc in range(4):
                    nc.tensor.matmul(
                        po[:],
                        lhsT=src_T[:, mc, qc * 128:(qc + 1) * 128],
                        rhs=Wo_p[:, mc, :],
                        start=(mc == 0), stop=False,
                    )
                nc.tensor.matmul(
                    po[:], lhsT=ones_row[:], rhs=bo_r[:], start=False, stop=True
                )
                ot = evac.tile([128, D], f32, tag="evac")
                nc.vector.tensor_copy(ot[:], po[:])
                nc.sync.dma_start(out_view[:, qc, :], ot[:])

        wo_project_packed(KTo, kp_v)    # Kp/Vp during the AG latency window
        wo_project_packed(VTo, vp_v)

        # V' readbacks: all four slices are consumed during every pass
        vp_s = []
        for j in range(4):
            t = vp_p.tile([128, N_CORES, H, E + 1], bf16, tag="vps",
                          name=f"vps{j}")
            nc.sync.dma_start(
                t[:].rearrange("p r h e -> p r (h e)"), ago_vp[j]
            )
            vp_s.append(t)

        # ---- phase 2: attention (4 passes x 2 heads) ------------------
        OT = single.tile([64, H, R], bf16)  # normalized outH^T
        W1_s = wpool.tile([128, 4, F], bf16)
        W2_s = wpool.tile([128, 4, 4, D], bf16)
        xro = single.tile([128, 4, D], f32)       # x own rows; becomes z

        kt_s = [None] * 4
        kt_s[0] = kt_p.tile([128, N_CORES, R], bf16, tag="kt", name="kt0")
        nc.sync.dma_start(kt_s[0][:], ago_kt[0])

        for pass_ in range(4):
            h0, h1 = 2 * pass_, 2 * pass_ + 1
            kt_t = kt_s[pass_]
            if pass_ + 1 < 4:
                kt_s[pass_ + 1] = kt_p.tile(
                    [128, N_CORES, R], bf16, tag="kt", name=f"kt{pass_ + 1}")
                nc.sync.dma_start(kt_s[pass_ + 1][:], ago_kt[pass_ + 1])
            if pass_ == 1:
                # stream the MLP weights and x own rows during attention
                nc.sync.dma_start(W1_s[:], w1_d)
                nc.sync.dma_start(W2_s[:], w2_d)
                nc.sync.dma_start(xro[:], xro_d)
            po_a = ps_po.tile([E + 1, R], f32, tag="po")
            po_b = ps_po.tile([E + 1, R], f32, tag="po")
            # AV runs one chunk behind the logits/exp stream so the in-order
            # PE queue never stalls on the current chunk's exp.
            pend = None
            for ci in range(32):
                tc, r = ci // 8, ci % 8   # slice-col outer: slice tc ready first
                pl = ps_mm.tile([128, 2, R], f32, tag="mm")
                nc.tensor.matmul(
                    pl[:, 0, :],
                    lhsT=kt_t[:, r, tc * 128:(tc + 1) * 128],
                    rhs=QT[:, h0, :], start=True, stop=True,
                )
                nc.tensor.matmul(
                    pl[:, 1, :],
                    lhsT=kt_t[:, r, tc * 128:(tc + 1) * 128],
                    rhs=QT[:, h1, :], start=True, stop=True,
                )
                pexp = pexp_p.tile([128, 2, R], bf16, tag="pexp")
                nc.scalar.activation(pexp[:], pl[:], AF.Exp, scale=SCALE)
                if pend is not None:
                    ptc, pr_, ppexp, pci = pend
                    nc.tensor.matmul(
                        po_a[:], lhsT=vp_s[ptc][:, pr_, h0, :],
                        rhs=ppexp[:, 0, :], start=(pci == 0), stop=False,
                    )
                    nc.tensor.matmul(
                        po_b[:], lhsT=vp_s[ptc][:, pr_, h1, :],
                        rhs=ppexp[:, 1, :], start=(pci == 0), stop=False,
                    )
                pend = (tc, r, pexp, ci)
            ptc, pr_, ppexp, pci = pend
            nc.tensor.matmul(
                po_a[:], lhsT=vp_s[ptc][:, pr_, h0, :], rhs=ppexp[:, 0, :],
                start=False, stop=True,
            )
            nc.tensor.matmul(
                po_b[:], lhsT=vp_s[ptc][:, pr_, h1, :], rhs=ppexp[:, 1, :],
                start=False, stop=True,
            )
            # normalize rows 0..63 by the ones-column row 64
            for po_t, h in ((po_a, h0), (po_b, h1)):
                otr = otr_p.tile([E + 1, R], f32, tag="otr")
                nc.vector.tensor_copy(otr[:], po_t[:])
                rden = otr_p.tile([1, R], f32r, tag="rden")
                nc.vector.reciprocal(rden[:], otr[E:E + 1, :])
                pb = ps_po.tile([E, R], f32, tag="po")
                nc.tensor.matmul(
                    pb[:], lhsT=ones_row_r[:, 0:E], rhs=rden[:],
                    start=True, stop=True,
                )
                nc.vector.tensor_tensor(OT[:, h, :], otr[0:E, :], pb[:], ALU.mult)

        # ---- phase 3: out proj + residual + global LN1 ----------------
        # Wo_s reuses a freed K^T slot (same pool tag, disjoint lifetime)
        Wo_s = kt_p.tile([64, H, D], bf16, tag="kt", name="Wo_s")
        nc.sync.dma_start(Wo_s[:], wos_d)
        z = xro  # in place: z = x + out
        for qc in range(4):
            po = ps_mm.tile([128, D], f32, tag="mm")
            for h in range(H):
                nc.tensor.matmul(
                    po[:],
                    lhsT=OT[:, h, qc * 128:(qc + 1) * 128],
                    rhs=Wo_s[:, h, :],
                    start=(h == 0), stop=False,
                )
            nc.tensor.matmul(
                po[:], lhsT=ones_row[:], rhs=bo_r[:], start=False, stop=True
            )
            nc.vector.tensor_tensor(z[:, qc, :], po[:], xro[:, qc, :], ALU.add)

        def stats_start(src_t, tag):
            """Partial [sum, sumsq] -> AllGather (cheaper floor than
            AllReduce for 8 B); returns the gathered [8, 2] dram tile."""
            sums = wk_p.tile([128, 5], f32, tag=f"sums{tag}")
            nc.vector.tensor_reduce(
                out=sums[:, 0:1], in_=src_t[:], axis=AX.XY, op=ALU.add
            )
            for qc in range(4):
                sqv = evac.tile([128, D], f32, tag="evac")
                nc.scalar.activation(
                    sqv[:], src_t[:, qc, :], AF.Square,
                    accum_out=sums[:, 1 + qc:2 + qc],
                )
            pr = ps_po.tile([1, 5], f32, tag="po")
            nc.tensor.matmul(
                pr[:], lhsT=onesP[:], rhs=sums[:], start=True, stop=True
            )
            part = wk_p.tile([1, 2], f32, tag=f"part{tag}")
            nc.vector.tensor_copy(part[:, 0:1], pr[:, 0:1])
            nc.vector.tensor_reduce(
                out=part[:, 1:2], in_=pr[:, 1:5], axis=AX.X, op=ALU.add
            )
            cin = dram.tile([1, 2], f32)
            cout = dram.tile([N_CORES, 2], f32, addr_space="Shared")
            nc.sync.dma_start(cin[:], part[:])
            nc.gpsimd.collective_compute(
                "AllGather", ALU.bypass,
                replica_groups=[list(range(N_CORES))],
                ins=[cin[:]], outs=[cout[:]],
            )
            return cout

        def stats_finish(cout, tag):
            """-> [128, 2] sbuf tile: [:,0]=rstd, [:,1]=-mu*rstd (global)."""
            tot8 = wk_p.tile([N_CORES, 2], f32, tag=f"tot8{tag}")
            nc.sync.dma_start(tot8[:], cout[:])
            pr8 = ps_po.tile([1, 2], f32, tag="po")
            nc.tensor.matmul(
                pr8[:], lhsT=onesP[0:N_CORES, :], rhs=tot8[:],
                start=True, stop=True,
            )
            tot = wk_p.tile([1, 2], f32, tag=f"tot{tag}")
            nc.vector.tensor_copy(tot[:], pr8[:])
            sc = wk_p.tile([1, 6], f32, tag=f"sc{tag}")
            mu, m2 = sc[0:1, 0:1], sc[0:1, 1:2]
            nc.vector.tensor_scalar_mul(mu, tot[0:1, 0:1], INV_SD)
            nc.vector.tensor_scalar_mul(m2, tot[0:1, 1:2], INV_SD)
            nc.vector.tensor_tensor(sc[0:1, 2:3], mu, mu, ALU.mult)
            nc.vector.tensor_tensor(sc[0:1, 3:4], m2, sc[0:1, 2:3], ALU.subtract)
            nc.scalar.activation(sc[0:1, 4:5], sc[0:1, 3:4], AF.Sqrt, bias=eps_t[:])
            st2 = wk_p.tile([1, 2], f32r, tag=f"st2{tag}")
            nc.vector.reciprocal(st2[0:1, 0:1], sc[0:1, 4:5])        # rstd
            nc.vector.tensor_tensor(sc[0:1, 5:6], mu, st2[0:1, 0:1], ALU.mult)
            nc.vector.tensor_scalar_mul(st2[0:1, 1:2], sc[0:1, 5:6], -1.0)
            pbc = ps_po.tile([128, 2], f32, tag="po")
            nc.tensor.matmul(pbc[:], lhsT=ones_row_r[:], rhs=st2[:],
                             start=True, stop=True)
            stb = wk_p.tile([128, 2], f32, tag=f"stb{tag}")
            nc.vector.tensor_copy(stb[:], pbc[:])
            return stb

        cout1 = stats_start(z, "a")
        # AR1 latency window: z^T transposes, then z^T @ W1 (the LN1 affine
        # commutes into the matmul in the fast_ln path)
        zT = single.tile([128, 4, R], bf16)        # z^T for the MLP path
        for dc in range(4):
            for qc in range(4):
                ptr = ps_po.tile([128, 128], f32, tag="po")
                nc.tensor.transpose(
                    ptr[:], z[:, qc, dc * 128:(dc + 1) * 128], ident[:]
                )
                nc.vector.tensor_copy(zT[:, dc, qc * 128:(qc + 1) * 128], ptr[:])

        h1T = single.tile([128, 16, R], bf16)
        if fast_ln:
            # zW1T = (z^T @ W1)^T in h1T's slot, relu-affine applied in place
            for fm in range(16):
                ph = ps_mm.tile([128, R], f32, tag="mm")
                for dc in range(4):
                    nc.tensor.matmul(
                        ph[:],
                        lhsT=W1_s[:, dc, fm * 128:(fm + 1) * 128],
                        rhs=zT[:, dc, :],
                        start=(dc == 0), stop=(dc == 3),
                    )
                nc.vector.tensor_copy(h1T[:, fm, :], ph[:])

        stb1 = stats_finish(cout1, "a")

        out1 = single.tile([128, 4, D], f32)
        if fast_ln:
            for qc in range(4):
                nc.scalar.activation(
                    out1[:, qc, :], z[:, qc, :], AF.Identity,
                    bias=stb1[:, 1:2], scale=stb1[:, 0:1],
                )
            # h1 = relu(rstd*zW1 + (b1 - mu*rstd*colsum(W1))) per f-partition
            bmlp = wk_p.tile([128, 16], f32, tag="bmlp")
            nc.vector.tensor_scalar(
                bmlp[:], cs1[:], stb1[0:128, 1:2], None, ALU.mult
            )
            nc.vector.tensor_tensor(bmlp[:], bmlp[:], b1s[:], ALU.add)
            for fm in range(16):
                nc.scalar.activation(
                    h1T[:, fm, :], h1T[:, fm, :], AF.Relu,
                    bias=bmlp[:, fm:fm + 1], scale=stb1[:, 0:1],
                )
        else:
            g_nat = single.tile([128, 4, D], bf16)
            nc.sync.dma_start(g_nat[:], gnat_d)
            b_nat = single.tile([128, 4, D], bf16)
            nc.sync.dma_start(b_nat[:], bnat_d)
            for qc in range(4):
                n_t = evac.tile([128, D], f32, tag="evac")
                nc.scalar.activation(
                    n_t[:], z[:, qc, :], AF.Identity,
                    bias=stb1[:, 1:2], scale=stb1[:, 0:1],
                )
                nc.vector.tensor_tensor(n_t[:], n_t[:], g_nat[:, qc, :], ALU.mult)
                nc.vector.tensor_tensor(
                    out1[:, qc, :], n_t[:], b_nat[:, qc, :], ALU.add)
            out1T = single.tile([128, 4, R], bf16)
            for dc in range(4):
                gT_t = evac.tile([128, R], bf16, tag="evacT")
                nc.sync.dma_start(gT_t[:], gT_d[:, dc, :])
                bT_t = evac.tile([128, R], bf16, tag="evacT")
                nc.sync.dma_start(bT_t[:], bT_d[:, dc, :])
                nT = evac.tile([128, R], bf16, tag="evacT")
                nc.scalar.activation(
                    nT[:], zT[:, dc, :], AF.Identity,
                    bias=stb1[:, 1:2], scale=stb1[:, 0:1],
                )
                nc.vector.tensor_tensor(nT[:], nT[:], gT_t[:], ALU.mult)
                nc.vector.tensor_tensor(out1T[:, dc, :], nT[:], bT_t[:], ALU.add)
            for fm in range(16):
                ph = ps_mm.tile([128, R], f32, tag="mm")
                for dc in range(4):
                    nc.tensor.matmul(
                        ph[:],
                        lhsT=W1_s[:, dc, fm * 128:(fm + 1) * 128],
                        rhs=out1T[:, dc, :],
                        start=(dc == 0), stop=(dc == 3),
                    )
                nc.scalar.activation(
                    h1T[:, fm, :], ph[:], AF.Relu, bias=b1s[:, fm:fm + 1]
                )

        # ---- phase 4: MLP second half + residual + global LN2 ---------
        w = out1  # in place: w = out1 + out2
        for qc in range(4):
            po = ps_mm.tile([128, D], f32, tag="mm")
            for fm in range(16):
                nc.tensor.matmul(
                    po[:],
                    lhsT=h1T[:, fm, qc * 128:(qc + 1) * 128],
                    rhs=W2_s[:, fm // 4, fm % 4, :],
                    start=(fm == 0), stop=False,
                )
            nc.tensor.matmul(
                po[:], lhsT=ones_row[:], rhs=b2_r[:], start=False, stop=True
            )
            nc.vector.tensor_tensor(w[:, qc, :], po[:], out1[:, qc, :], ALU.add)

        cout2 = stats_start(w, "b")
        stb2 = stats_finish(cout2, "b")
        for qc in range(4):
            n_t = evac.tile([128, D], f32, tag="evac")
            nc.scalar.activation(
                n_t[:], w[:, qc, :], AF.Identity,
                bias=stb2[:, 1:2], scale=stb2[:, 0:1],
            )
            if not fast_ln:
                nc.vector.tensor_tensor(n_t[:], n_t[:], g_nat[:, qc, :], ALU.mult)
                nc.vector.tensor_tensor(n_t[:], n_t[:], b_nat[:, qc, :], ALU.add)
            nc.sync.dma_start(fin_v[:, qc, :], n_t[:])

    split_waits(nc)
    return nc


def _prep(inp, fast_ln):
    """Host-side layout prep: cast weights to bf16 and pre-arrange into the
    exact SBUF layouts the kernel uses."""
    f = {k: np.ascontiguousarray(np.asarray(v, dtype=np.float32))
         for k, v in inp.items()}

    def tile128(a):  # [(c 128), n] -> [128, c, n]
        c = a.shape[0] // 128
        return np.ascontiguousarray(
            a.reshape(c, 128, a.shape[1]).transpose(1, 0, 2))

    def pack_heads(w):  # [H, D, E] -> [D, 512] with he = (h//2)*128+(h%2)*64+e
        out = np.zeros((D, D), np.float32)
        for h in range(H):
            out[:, (h // 2) * 128 + (h % 2) * 64:
                (h // 2) * 128 + (h % 2) * 64 + E] = w[h]
        return out

    def pack2(b):  # [H, E] -> [128, 4], p = (h%2)*64+e
        return np.ascontiguousarray(
            b.reshape(4, 2, E).transpose(1, 2, 0).reshape(128, 4))

    shared = dict(
        wq=tile128(pack_heads(f["Wq"])).astype(BF16),
        wk=tile128(pack_heads(f["Wk"])).astype(BF16),
        wv=tile128(pack_heads(f["Wv"])).astype(BF16),
        wo_s=np.ascontiguousarray(
            f["Wo"].reshape(H, E, D).transpose(1, 0, 2)).astype(BF16),
        wo_p=np.ascontiguousarray(
            f["Wo"].reshape(4, 2, E, D).transpose(1, 2, 0, 3)
            .reshape(128, 4, D)).astype(BF16),
        w1=tile128(f["W1"]).astype(BF16),
        w2=np.ascontiguousarray(
            f["W2"].reshape(4, 4, 128, D).transpose(2, 0, 1, 3)).astype(BF16),
        colsum_w1=np.ascontiguousarray(
            f["W1"].astype(BF16).astype(np.float32).sum(0)
            .reshape(16, 128).T),
        bqs2=pack2(f["bq"]),
        bks2=pack2(f["bk"]),
        bvs2=pack2(f["bv"]),
        bv_bc=np.ascontiguousarray(np.tile(f["bv"].reshape(1, D), (128, 1))),
        b1s=np.ascontiguousarray(f["b1"].reshape(16, 128).T),
        bo_r=f["bo"].reshape(1, D).astype(BF16),
        b2_r=f["b2"].reshape(1, D).astype(BF16),
    )

    def per_core(c):
        rows = slice(c * R, (c + 1) * R)
        xr = f["x"][rows]
        m = dict(
            xro=tile128(xr),
            xrT=tile128(np.ascontiguousarray(xr.T)).astype(BF16),
            **shared,
        )
        if not fast_ln:
            m.update(
                g_nat=tile128(f["ln_g"][rows]).astype(BF16),
                b_nat=tile128(f["ln_b"][rows]).astype(BF16),
                gT=tile128(np.ascontiguousarray(f["ln_g"][rows].T)).astype(BF16),
                bT=tile128(np.ascontiguousarray(f["ln_b"][rows].T)).astype(BF16),
            )
        return m

    return [per_core(c) for c in range(N_CORES)]


_NC_CACHE = {}


def _fast_ln_ok(inputs):
    return bool(
        np.all(np.asarray(inputs["ln_g"]) == 1.0)
        and np.all(np.asarray(inputs["ln_b"]) == 0.0)
    )


def _get_nc(fast_ln=True):
    if fast_ln not in _NC_CACHE:
        _NC_CACHE[fast_ln] = build_nc(fast_ln)
    return _NC_CACHE[fast_ln]


def make_in_maps(inputs):
    return _prep(inputs, _fast_ln_ok(inputs))


def kernel(**inputs):
    fast_ln = _fast_ln_ok(inputs)
    in_maps = _prep(inputs, fast_ln)
    nc = _get_nc(fast_ln)
    res = run_bass_kernel_spmd(nc, in_maps, list(range(N_CORES)))
    final = np.concatenate([res.results[c]["final_rows"] for c in range(N_CORES)])
    Kp = np.concatenate([res.results[c]["Kp_rows"] for c in range(N_CORES)])
    Vp = np.concatenate([res.results[c]["Vp_rows"] for c in range(N_CORES)])
    return (final, Kp, Vp)
